# revision 26
# baseline (speedup 1.0000x reference)
"""DynamicDeepHit (GRU + additive attention + cause MLP) Bass kernel for 8 trn2 cores.

Sharding: pure data-parallel over batch B=1024 -> 128 per core; weights replicated.

Per-core device program (SPMD, identical on all 8 cores):
  Loop over L=256 steps (fully unrolled), batch-major layout [B=128 part, feat]:
    - x_t^T loaded pre-transposed from DRAM ([L, F, B] host layout)
    - G = x@W_ihT + h@W_hhT via PE with lhsT = x^T / h^T (stationary), accum in PSUM
    - r = sigmoid(Gr); z = sigmoid(Gz + BIG*(1-active_t))  (freeze folded into z)
    - n = tanh(Gni + r*Gnh)
    - h = z*h_old + (1-z)*n  (bf16, DVE + GPSIMD)
    - h^T via 2 PE transposes + one PSUM->SBUF copy
    - attention score s_t = tanh(h@Wa + last@Ua) . va  (PE + ACT + fused mul-reduce)
    - out_t = active_t * (h@W_outT)  -> DMA to DRAM
    - h_t kept in an SBUF ring (bf16) for the deferred context accumulation
  Post-loop: E = exp(scores); e' = E*mask; d = sum(e') + pad_d (host-computed padded
  softmax mass); context = (sum_t e'_t * h_t) / d; MLP + softmax -> fht.

PSUM budget (8 banks, one accumulation group per bank at a time):
  grz x2 (double-buffered), gni, gnh, s1, o, hTp, acc.
"""

import numpy as np
import sys

for _p in ("/opt/trn_rl_repo", "/root/.axon_site/_ro/trn_rl_repo"):
    if _p not in sys.path:
        sys.path.insert(0, _p)

import ml_dtypes

import concourse.bacc as bacc
import concourse.bass as bass
import concourse.tile as tile
from concourse import mybir
from concourse.bass_utils import run_bass_kernel_spmd

BF16 = ml_dtypes.bfloat16

# Problem dims (hardcoded per spec)
B, L, F, H = 1024, 256, 128, 256
A, CH, OUT = 128, 256, 512
NCORES = 8
BL = B // NCORES  # 128 per core
BIG = 30.0

f32 = mybir.dt.float32
bf16 = mybir.dt.bfloat16
AF = mybir.ActivationFunctionType
ALU = mybir.AluOpType

_CACHE = {}


def _build(has_gbias: bool, has_obias: bool):
    key = (has_gbias, has_obias)
    if key in _CACHE:
        return _CACHE[key]

    nc = bacc.Bacc("TRN2", target_bir_lowering=False, debug=False,
                   num_devices=NCORES)

    # ---- DRAM I/O (per-core shapes) ----
    d_xT = nc.dram_tensor("xT", [L, F, BL], bf16, kind="ExternalInput")
    d_wih = nc.dram_tensor("wih", [F, 3 * H], bf16, kind="ExternalInput")
    d_whh = nc.dram_tensor("whh", [2, 128, 3 * H], bf16, kind="ExternalInput")
    d_wa = nc.dram_tensor("wa", [128, 2 * A], bf16, kind="ExternalInput")
    d_woutT = nc.dram_tensor("woutT", [128, 2 * F], bf16, kind="ExternalInput")
    d_lastua = nc.dram_tensor("lastua", [BL, A], bf16, kind="ExternalInput")
    d_vab = nc.dram_tensor("vab", [BL, A], bf16, kind="ExternalInput")
    d_mask = nc.dram_tensor("mask", [BL, L], f32, kind="ExternalInput")
    d_zbias = nc.dram_tensor("zbias", [BL, L], f32, kind="ExternalInput")
    d_padd = nc.dram_tensor("padd", [BL, 1], f32, kind="ExternalInput")
    d_lastT = nc.dram_tensor("lastT", [F, BL], f32, kind="ExternalInput")
    d_w1 = nc.dram_tensor("w1p", [128, 3 * CH], f32, kind="ExternalInput")
    d_w2 = nc.dram_tensor("w2p", [128, 2 * OUT], f32, kind="ExternalInput")
    d_b1 = nc.dram_tensor("b1r", [1, CH], f32, kind="ExternalInput")
    d_b2 = nc.dram_tensor("b2r", [1, OUT], f32, kind="ExternalInput")
    d_eye16 = nc.dram_tensor("eye16", [128, 128], bf16, kind="ExternalInput")
    d_eye32 = nc.dram_tensor("eye32", [128, 128], f32, kind="ExternalInput")
    if has_gbias:
        d_gbias = nc.dram_tensor("gbias", [1, 3 * H], f32, kind="ExternalInput")
    if has_obias:
        d_obias = nc.dram_tensor("obias", [1, F], f32, kind="ExternalInput")

    d_hid = nc.dram_tensor("hid", [L, BL, H], f32)
    d_out = nc.dram_tensor("out", [BL, L, F], f32, kind="ExternalOutput")
    d_fht = nc.dram_tensor("fht", [BL, OUT], f32, kind="ExternalOutput")

    with tile.TileContext(nc) as tc:
        with (
            tc.tile_pool(name="const", bufs=1) as cp,
            tc.tile_pool(name="hwork", bufs=3) as hwork,
            tc.tile_pool(name="hback", bufs=8) as hback,
            tc.tile_pool(name="work", bufs=3) as wk,
            tc.tile_pool(name="xin", bufs=6) as xin,
            tc.tile_pool(name="ostage", bufs=6) as ost,
            tc.tile_pool(name="psGrz", bufs=2, space="PSUM") as psGrz,
            tc.tile_pool(name="psGni", bufs=1, space="PSUM") as psGni,
            tc.tile_pool(name="psGnh", bufs=1, space="PSUM") as psGnh,
            tc.tile_pool(name="psS1", bufs=1, space="PSUM") as psS1,
            tc.tile_pool(name="psO", bufs=1, space="PSUM") as psO,
            tc.tile_pool(name="psT", bufs=1, space="PSUM") as psT,
            tc.tile_pool(name="psAcc", bufs=1, space="PSUM") as psAcc,
        ):
            # ---- constants into SBUF ----
            wih = cp.tile([F, 3 * H], bf16, tag="wih")
            nc.sync.dma_start(out=wih, in_=d_wih[:])
            whh0 = cp.tile([128, 3 * H], bf16, tag="whh0")
            nc.sync.dma_start(out=whh0, in_=d_whh[0])
            whh1 = cp.tile([128, 3 * H], bf16, tag="whh1")
            nc.sync.dma_start(out=whh1, in_=d_whh[1])
            wa = cp.tile([128, 2 * A], bf16, tag="wa")
            nc.sync.dma_start(out=wa, in_=d_wa[:])
            woutT = cp.tile([128, 2 * F], bf16, tag="woutT")
            nc.sync.dma_start(out=woutT, in_=d_woutT[:])
            lastua = cp.tile([BL, A], bf16, tag="lastua")
            nc.sync.dma_start(out=lastua, in_=d_lastua[:])
            vab = cp.tile([BL, A], bf16, tag="vab")
            nc.sync.dma_start(out=vab, in_=d_vab[:])
            maskt = cp.tile([BL, L], f32, tag="mask")
            nc.sync.dma_start(out=maskt, in_=d_mask[:])
            zbias = cp.tile([BL, L], f32, tag="zbias")
            nc.sync.dma_start(out=zbias, in_=d_zbias[:])
            padd = cp.tile([BL, 1], f32, tag="padd")
            nc.sync.dma_start(out=padd, in_=d_padd[:])
            lastT = cp.tile([F, BL], f32, tag="lastT")
            nc.sync.dma_start(out=lastT, in_=d_lastT[:])
            w1p = cp.tile([128, 3 * CH], f32, tag="w1p")
            nc.sync.dma_start(out=w1p, in_=d_w1[:])
            w2p = cp.tile([128, 2 * OUT], f32, tag="w2p")
            nc.sync.dma_start(out=w2p, in_=d_w2[:])
            b1r = cp.tile([1, CH], f32, tag="b1r")
            nc.sync.dma_start(out=b1r, in_=d_b1[:])
            b2r = cp.tile([1, OUT], f32, tag="b2r")
            nc.sync.dma_start(out=b2r, in_=d_b2[:])
            eye16 = cp.tile([128, 128], bf16, tag="eye16")
            nc.sync.dma_start(out=eye16, in_=d_eye16[:])
            eye32 = cp.tile([128, 128], f32, tag="eye32")
            nc.sync.dma_start(out=eye32, in_=d_eye32[:])
            ones1 = cp.tile([1, 128], f32, tag="ones1")
            nc.vector.memset(ones1, 1.0)
            onesb = cp.tile([BL, H], bf16, tag="onesb")
            nc.vector.memset(onesb, 1.0)
            if has_gbias:
                gbias = cp.tile([1, 3 * H], f32, tag="gbias")
                nc.sync.dma_start(out=gbias, in_=d_gbias[:])
            if has_obias:
                obias = cp.tile([1, F], f32, tag="obias")
                nc.sync.dma_start(out=obias, in_=d_obias[:])

            scores = cp.tile([BL, L], f32, tag="scores")

            # ---- GRU loop ----
            # Emission order per iteration: G-matmuls for step t first (so the
            # PE queue prioritizes the recurrence), then the deferred
            # attention/output block for step t-1, then gate math for t.
            hT = None
            h_prev = None

            def attn_out_block(t, hT_t):
                # S1 = lastUa + h@Wa ; s_t = tanh(S1).va ; out_t = mask*(h@WoutT)
                s1 = psS1.tile([BL, A], f32, tag="s1", name=f"s1_{t}")
                nc.tensor.matmul(s1, eye16, lastua, start=True, stop=False)
                nc.tensor.matmul(s1, hT_t[:, 0:128], wa[:, 0:A],
                                 start=False, stop=False)
                nc.tensor.matmul(s1, hT_t[:, 128:256], wa[:, A:2 * A],
                                 start=False, stop=True)
                o = psO.tile([BL, F], f32, tag="o", name=f"o_{t}")
                nc.tensor.matmul(o, hT_t[:, 0:128], woutT[:, 0:F],
                                 start=True, stop=False)
                nc.tensor.matmul(o, hT_t[:, 128:256], woutT[:, F:2 * F],
                                 start=False, stop=not has_obias)
                if has_obias:
                    nc.tensor.matmul(o, ones1[:, 0:BL], obias,
                                     start=False, stop=True)
                tanha = wk.tile([BL, A], bf16, tag="tanha", name=f"tanha_{t}")
                nc.scalar.activation(tanha, s1, AF.Tanh)
                scratch = wk.tile([BL, A], bf16, tag="scratch", name=f"scr_{t}")
                nc.vector.scalar_tensor_tensor(
                    scratch, tanha, 1.0, vab, op0=ALU.mult, op1=ALU.mult,
                    accum_out=scores[:, t:t + 1])
                ot = ost.tile([BL, F], f32, tag="ot", name=f"ot_{t}")
                nc.vector.tensor_scalar_mul(ot, o, maskt[:, t:t + 1])
                nc.sync.dma_start(out=d_out[:, t, :], in_=ot)

            for t in range(L):
                xt = xin.tile([F, BL], bf16, tag="xt")
                nc.sync.dma_start(out=xt, in_=d_xT[t])

                grz = psGrz.tile([BL, 2 * H], f32, tag="grz")
                gni = psGni.tile([BL, H], f32, tag="gni")
                if t > 0:
                    gnh = psGnh.tile([BL, H], f32, tag="gnh")
                else:
                    gnh = None

                # x contributions first (no hT dependency; PE can run them early)
                nc.tensor.matmul(grz, xt, wih[:, 0:2 * H], start=True,
                                 stop=(t == 0) and not has_gbias)
                gni_instant = (t == 0) and not has_gbias
                nc.tensor.matmul(gni, xt, wih[:, 2 * H:3 * H],
                                 start=True, stop=gni_instant)
                if has_gbias:
                    nc.tensor.matmul(grz, ones1[:, 0:BL], gbias[:, 0:2 * H],
                                     start=False, stop=(t == 0))
                    nc.tensor.matmul(gni, ones1[:, 0:BL], gbias[:, 2 * H:3 * H],
                                     start=False, stop=(t == 0))
                # h contributions
                if t > 0:
                    nc.tensor.matmul(grz, hT[:, 0:128], whh0[:, 0:2 * H],
                                     start=False, stop=False)
                    nc.tensor.matmul(grz, hT[:, 128:256], whh1[:, 0:2 * H],
                                     start=False, stop=True)
                    nc.tensor.matmul(gnh, hT[:, 0:128], whh0[:, 2 * H:3 * H],
                                     start=True, stop=False)
                    nc.tensor.matmul(gnh, hT[:, 128:256], whh1[:, 2 * H:3 * H],
                                     start=False, stop=True)

                r = wk.tile([BL, H], bf16, tag="r")
                nc.scalar.activation(r, grz[:, 0:H], AF.Sigmoid)
                z = wk.tile([BL, H], bf16, tag="z")
                nc.scalar.activation(z, grz[:, H:2 * H], AF.Sigmoid,
                                     bias=zbias[:, t:t + 1])
                zb = wk.tile([BL, H], bf16, tag="zb")
                nc.vector.scalar_tensor_tensor(zb, z, -1.0, onesb,
                                               op0=ALU.mult, op1=ALU.add)
                hTp = psT.tile([128, 2 * BL], f32, tag="hTp")
                if t > 0:
                    w1t = wk.tile([BL, H], f32, tag="w1t")
                    nc.gpsimd.tensor_mul(w1t, z, h_prev)
                    # transpose w1t into the hT accumulator early (off-chain)
                    nc.tensor.matmul(hTp[:, 0:BL], w1t[:, 0:128], eye32,
                                     is_transpose=True, start=True, stop=False)
                    nc.tensor.matmul(hTp[:, BL:2 * BL], w1t[:, 128:256], eye32,
                                     is_transpose=True, start=False, stop=False)

                n = wk.tile([BL, H], bf16, tag="n")
                if t > 0:
                    gnhs = wk.tile([BL, H], bf16, tag="gnhs")
                    nc.vector.tensor_copy(gnhs, gnh)
                    u = wk.tile([BL, H], bf16, tag="u")
                    nc.vector.tensor_mul(u, r, gnhs)
                    nc.tensor.matmul(gni, eye16, u, start=False, stop=True)
                nc.scalar.activation(n, gni, AF.Tanh)

                w2t = wk.tile([BL, H], f32, tag="w2t")
                nc.vector.tensor_mul(w2t, zb, n)
                # h^T = T(w1t) + T(w2t) accumulated on PE
                nc.tensor.matmul(hTp[:, 0:BL], w2t[:, 0:128], eye32,
                                 is_transpose=True, start=(t == 0), stop=False)
                nc.tensor.matmul(hTp[:, BL:2 * BL], w2t[:, 128:256], eye32,
                                 is_transpose=True, start=False, stop=True)
                hT = wk.tile([128, 2 * BL], bf16, tag="hT")
                nc.vector.tensor_copy(hT[:, 0:BL], hTp[:, 0:BL])
                nc.scalar.copy(hT[:, BL:2 * BL], hTp[:, BL:2 * BL])

                # h natural (off the recurrence chain): hid DMA + next-step w1t
                h = hwork.tile([BL, H], f32, tag="h")
                if t > 0:
                    nc.vector.tensor_add(h, w1t, w2t)
                else:
                    nc.vector.tensor_copy(h, w2t)
                h_prev = h
                nc.sync.dma_start(out=d_hid[t], in_=h)

                # deferred attention/output block for the previous step
                if t > 0:
                    attn_out_block(t - 1, hT_prev_attn)
                hT_prev_attn = hT


            attn_out_block(L - 1, hT)

            # ---- deferred attention context ----
            Et = cp.tile([BL, L], f32, tag="Et")
            nc.scalar.activation(Et, scores, AF.Exp)
            ep = cp.tile([BL, L], f32, tag="ep")
            nc.vector.tensor_mul(ep, Et, maskt)
            d0 = cp.tile([BL, 1], f32, tag="d0")
            nc.vector.tensor_reduce(d0, ep, axis=mybir.AxisListType.X, op=ALU.add)
            dsum = cp.tile([BL, 1], f32, tag="dsum")
            nc.vector.tensor_add(dsum, d0, padd)
            rd = cp.tile([BL, 1], f32, tag="rd")
            nc.vector.reciprocal(rd, dsum)

            acc = psAcc.tile([BL, H], f32, tag="acc")
            for t in range(L):
                hld = hback.tile([BL, H], f32, tag="hld")
                nc.sync.dma_start(out=hld, in_=d_hid[t])
                tmp = wk.tile([BL, H], bf16, tag="tmp")
                nc.vector.tensor_scalar_mul(tmp, hld, ep[:, t:t + 1])
                nc.tensor.matmul(acc, eye16, tmp, start=(t == 0), stop=(t == L - 1))

            ctx = cp.tile([BL, H], f32, tag="ctx")
            nc.vector.tensor_scalar_mul(ctx, acc, rd)

            # ---- cause MLP ----  (PSUM tiles reuse loop tags to stay in 8 banks)
            ctxTp = psS1.tile([128, 2 * BL], f32, tag="s1")
            nc.tensor.transpose(ctxTp[:, 0:BL], ctx[:, 0:128], eye32)
            nc.tensor.transpose(ctxTp[:, BL:2 * BL], ctx[:, 128:256], eye32)
            ctxT = cp.tile([128, 2 * BL], f32, tag="ctxT")
            nc.vector.tensor_copy(ctxT, ctxTp)

            zcp = psGrz.tile([BL, CH], f32, tag="grz")
            nc.tensor.matmul(zcp, ctxT[:, 0:BL], w1p[:, 0:CH], start=True, stop=False)
            nc.tensor.matmul(zcp, ctxT[:, BL:2 * BL], w1p[:, CH:2 * CH],
                             start=False, stop=False)
            nc.tensor.matmul(zcp, lastT, w1p[:, 2 * CH:3 * CH], start=False, stop=False)
            nc.tensor.matmul(zcp, ones1[:, 0:BL], b1r, start=False, stop=True)
            zc = cp.tile([BL, CH], f32, tag="zc")
            nc.scalar.activation(zc, zcp, AF.Relu)

            zcTp = psS1.tile([128, 2 * BL], f32, tag="s1")
            nc.tensor.transpose(zcTp[:, 0:BL], zc[:, 0:128], eye32)
            nc.tensor.transpose(zcTp[:, BL:2 * BL], zc[:, 128:256], eye32)
            zcT = cp.tile([128, 2 * BL], f32, tag="zcT")
            nc.vector.tensor_copy(zcT, zcTp)

            lg = psGrz.tile([BL, OUT], f32, tag="grz")
            nc.tensor.matmul(lg, zcT[:, 0:BL], w2p[:, 0:OUT], start=True, stop=False)
            nc.tensor.matmul(lg, zcT[:, BL:2 * BL], w2p[:, OUT:2 * OUT],
                             start=False, stop=False)
            nc.tensor.matmul(lg, ones1[:, 0:BL], b2r, start=False, stop=True)

            mx = cp.tile([BL, 1], f32, tag="mx")
            nc.vector.tensor_reduce(mx, lg, axis=mybir.AxisListType.X, op=ALU.max)
            nmx = cp.tile([BL, 1], f32, tag="nmx")
            nc.vector.tensor_scalar_mul(nmx, mx, -1.0)
            ex = cp.tile([BL, OUT], f32, tag="ex")
            sm = cp.tile([BL, 1], f32, tag="sm")
            nc.scalar.activation(ex, lg, AF.Exp, bias=nmx, accum_out=sm)
            rs = cp.tile([BL, 1], f32, tag="rs")
            nc.vector.reciprocal(rs, sm)
            fht = cp.tile([BL, OUT], f32, tag="fht")
            nc.vector.tensor_scalar_mul(fht, ex, rs)
            nc.sync.dma_start(out=d_fht[:], in_=fht)

    nc.finalize()
    _CACHE[key] = nc
    return nc


def _prep(input_batch, tte, W_ih, W_hh, b_ih, b_hh, W_out, b_out,
          Wa, Ua, va, W1, b1, W2, b2):
    input_batch = np.asarray(input_batch, np.float32)
    tte = np.asarray(tte, np.int32)
    W_ih = np.asarray(W_ih, np.float32)
    W_hh = np.asarray(W_hh, np.float32)
    b_ih = np.asarray(b_ih, np.float32)
    b_hh = np.asarray(b_hh, np.float32)
    W_out = np.asarray(W_out, np.float32)
    b_out = np.asarray(b_out, np.float32)
    Wa_ = np.asarray(Wa, np.float32)
    Ua = np.asarray(Ua, np.float32)
    va = np.asarray(va, np.float32)
    W1 = np.asarray(W1, np.float32)
    b1 = np.asarray(b1, np.float32)
    W2 = np.asarray(W2, np.float32)
    b2 = np.asarray(b2, np.float32)

    gb = b_ih + b_hh
    has_gbias = bool(np.any(gb))
    has_obias = bool(np.any(b_out))

    # shared (replicated) tensors
    wih_np = np.ascontiguousarray(W_ih.T).astype(BF16)              # [F, 3H]
    WhhT = W_hh.T                                                   # [H, 3H]
    whh_np = np.stack([WhhT[0:128], WhhT[128:256]]).astype(BF16)    # [2,128,3H]
    wa_np = np.concatenate([Wa_[0:128], Wa_[128:256]], axis=1).astype(BF16)
    WoutT = W_out.T                                                 # [H, F]
    woutT_np = np.concatenate([WoutT[0:128], WoutT[128:256]], axis=1).astype(BF16)
    w1_np = np.concatenate([W1[0:128], W1[128:256], W1[256:384]],
                           axis=1).astype(np.float32)               # [128, 3CH]
    w2_np = np.concatenate([W2[0:128], W2[128:256]], axis=1).astype(np.float32)
    b1_np = b1[None, :].astype(np.float32)
    b2_np = b2[None, :].astype(np.float32)
    eye16_np = np.eye(128, dtype=np.float32).astype(BF16)
    eye32_np = np.eye(128, dtype=np.float32)
    vab_np = np.broadcast_to(va, (BL, A)).astype(BF16).copy()
    gb_np = gb[None, :].astype(np.float32)
    ob_np = b_out[None, :].astype(np.float32)

    t_idx = np.arange(L, dtype=np.int32)[None, :]
    in_maps = []
    for c in range(NCORES):
        sl = slice(c * BL, (c + 1) * BL)
        xb = input_batch[sl]                      # [BL, L, F]
        ttec = tte[sl]
        last = xb[np.arange(BL), ttec]            # [BL, F]
        lastUa = last @ Ua                        # [BL, A]
        c_pad = np.tanh(lastUa) @ va              # [BL]
        pad_d = ((L - ttec).astype(np.float32) * np.exp(c_pad)).astype(np.float32)
        active = (t_idx < ttec[:, None]).astype(np.float32)  # [BL, L]
        m = {
            "xT": np.ascontiguousarray(xb.transpose(1, 2, 0)).astype(BF16),
            "wih": wih_np, "whh": whh_np, "wa": wa_np, "woutT": woutT_np,
            "lastua": lastUa.astype(BF16), "vab": vab_np,
            "mask": active, "zbias": (BIG * (1.0 - active)).astype(np.float32),
            "padd": pad_d[:, None],
            "lastT": np.ascontiguousarray(last.T).astype(np.float32),
            "w1p": w1_np, "w2p": w2_np, "b1r": b1_np, "b2r": b2_np,
            "eye16": eye16_np, "eye32": eye32_np,
        }
        if has_gbias:
            m["gbias"] = gb_np
        if has_obias:
            m["obias"] = ob_np
        in_maps.append(m)

    return in_maps, has_gbias, has_obias


def _prep_inputs(inputs):
    """Host-side preprocessing: returns (in_maps, has_gbias, has_obias)."""
    return _prep(**inputs)


def kernel(**inputs):
    in_maps, has_gbias, has_obias = _prep(**inputs)
    nc = _build(has_gbias, has_obias)
    res = run_bass_kernel_spmd(nc, in_maps, core_ids=list(range(NCORES)))
    outs = res.results
    output_batch = np.concatenate([r["out"] for r in outs], axis=0)
    fht = np.concatenate([r["fht"] for r in outs], axis=0)
    return output_batch.astype(np.float32), fht.astype(np.float32)


# revision 31
# speedup vs baseline: 1.0620x; 1.0620x over previous
"""DynamicDeepHit (GRU + additive attention + cause MLP) Bass kernel for 8 trn2 cores.

Sharding: pure data-parallel over batch B=1024 -> 128 per core; weights replicated.

Per-core device program (SPMD, identical on all 8 cores):
  Loop over L=256 steps (fully unrolled), batch-major layout [B=128 part, feat]:
    - x_t^T loaded pre-transposed from DRAM ([L, F, B] host layout)
    - G = x@W_ihT + h@W_hhT via PE with lhsT = x^T / h^T (stationary), accum in PSUM
    - r = sigmoid(Gr); z = sigmoid(Gz + BIG*(1-active_t))  (freeze folded into z)
    - n = tanh(Gni + r*Gnh)
    - h = z*h_old + (1-z)*n  (bf16, DVE + GPSIMD)
    - h^T via 2 PE transposes + one PSUM->SBUF copy
    - attention score s_t = tanh(h@Wa + last@Ua) . va  (PE + ACT + fused mul-reduce)
    - out_t = active_t * (h@W_outT)  -> DMA to DRAM
    - h_t kept in an SBUF ring (bf16) for the deferred context accumulation
  Post-loop: E = exp(scores); e' = E*mask; d = sum(e') + pad_d (host-computed padded
  softmax mass); context = (sum_t e'_t * h_t) / d; MLP + softmax -> fht.

PSUM budget (8 banks, one accumulation group per bank at a time):
  grz x2 (double-buffered), gni, gnh, s1, o, hTp, acc.
"""

import numpy as np
import sys

for _p in ("/opt/trn_rl_repo", "/root/.axon_site/_ro/trn_rl_repo"):
    if _p not in sys.path:
        sys.path.insert(0, _p)

import ml_dtypes

import concourse.bacc as bacc
import concourse.bass as bass
import concourse.tile as tile
from concourse import mybir
from concourse.bass_utils import run_bass_kernel_spmd

BF16 = ml_dtypes.bfloat16

# Problem dims (hardcoded per spec)
B, L, F, H = 1024, 256, 128, 256
A, CH, OUT = 128, 256, 512
NCORES = 8
BL = B // NCORES  # 128 per core
BIG = 30.0

f32 = mybir.dt.float32
bf16 = mybir.dt.bfloat16
AF = mybir.ActivationFunctionType
ALU = mybir.AluOpType

_CACHE = {}


def _build(has_gbias: bool, has_obias: bool):
    key = (has_gbias, has_obias)
    if key in _CACHE:
        return _CACHE[key]

    nc = bacc.Bacc("TRN2", target_bir_lowering=False, debug=False,
                   num_devices=NCORES)

    # ---- DRAM I/O (per-core shapes) ----
    d_xT = nc.dram_tensor("xT", [L, F, BL], bf16, kind="ExternalInput")
    d_wih = nc.dram_tensor("wih", [F, 3 * H], bf16, kind="ExternalInput")
    d_whh = nc.dram_tensor("whh", [2, 128, 3 * H], bf16, kind="ExternalInput")
    d_wa = nc.dram_tensor("wa", [128, 2 * A], bf16, kind="ExternalInput")
    d_woutT = nc.dram_tensor("woutT", [128, 2 * F], bf16, kind="ExternalInput")
    d_lastua = nc.dram_tensor("lastua", [BL, A], bf16, kind="ExternalInput")
    d_vab = nc.dram_tensor("vab", [BL, A], bf16, kind="ExternalInput")
    d_mask = nc.dram_tensor("mask", [BL, L], f32, kind="ExternalInput")
    d_zbias = nc.dram_tensor("zbias", [BL, L], f32, kind="ExternalInput")
    d_padd = nc.dram_tensor("padd", [BL, 1], f32, kind="ExternalInput")
    d_lastT = nc.dram_tensor("lastT", [F, BL], f32, kind="ExternalInput")
    d_w1 = nc.dram_tensor("w1p", [128, 3 * CH], f32, kind="ExternalInput")
    d_w2 = nc.dram_tensor("w2p", [128, 2 * OUT], f32, kind="ExternalInput")
    d_b1 = nc.dram_tensor("b1r", [1, CH], f32, kind="ExternalInput")
    d_b2 = nc.dram_tensor("b2r", [1, OUT], f32, kind="ExternalInput")
    d_eye16 = nc.dram_tensor("eye16", [128, 128], bf16, kind="ExternalInput")
    d_eye32 = nc.dram_tensor("eye32", [128, 128], f32, kind="ExternalInput")
    if has_gbias:
        d_gbias = nc.dram_tensor("gbias", [1, 3 * H], f32, kind="ExternalInput")
    if has_obias:
        d_obias = nc.dram_tensor("obias", [1, F], f32, kind="ExternalInput")

    d_hid = nc.dram_tensor("hid", [L, BL, H], f32)
    d_out = nc.dram_tensor("out", [BL, L, F], f32, kind="ExternalOutput")
    d_fht = nc.dram_tensor("fht", [BL, OUT], f32, kind="ExternalOutput")

    with tile.TileContext(nc) as tc:
        with (
            tc.tile_pool(name="const", bufs=1) as cp,
            tc.tile_pool(name="hwork", bufs=4) as hwork,
            tc.tile_pool(name="hback", bufs=8) as hback,
            tc.tile_pool(name="work", bufs=4) as wk,
            tc.tile_pool(name="xin", bufs=6) as xin,
            tc.tile_pool(name="ostage", bufs=6) as ost,
            tc.tile_pool(name="psGrz", bufs=2, space="PSUM") as psGrz,
            tc.tile_pool(name="psGni", bufs=1, space="PSUM") as psGni,
            tc.tile_pool(name="psGnh", bufs=1, space="PSUM") as psGnh,
            tc.tile_pool(name="psS1", bufs=1, space="PSUM") as psS1,
            tc.tile_pool(name="psO", bufs=1, space="PSUM") as psO,
            tc.tile_pool(name="psT", bufs=1, space="PSUM") as psT,
            tc.tile_pool(name="psAcc", bufs=1, space="PSUM") as psAcc,
        ):
            # ---- constants into SBUF ----
            wih = cp.tile([F, 3 * H], bf16, tag="wih")
            nc.sync.dma_start(out=wih, in_=d_wih[:])
            whh0 = cp.tile([128, 3 * H], bf16, tag="whh0")
            nc.sync.dma_start(out=whh0, in_=d_whh[0])
            whh1 = cp.tile([128, 3 * H], bf16, tag="whh1")
            nc.sync.dma_start(out=whh1, in_=d_whh[1])
            wa = cp.tile([128, 2 * A], bf16, tag="wa")
            nc.sync.dma_start(out=wa, in_=d_wa[:])
            woutT = cp.tile([128, 2 * F], bf16, tag="woutT")
            nc.sync.dma_start(out=woutT, in_=d_woutT[:])
            lastua = cp.tile([BL, A], bf16, tag="lastua")
            nc.sync.dma_start(out=lastua, in_=d_lastua[:])
            vab = cp.tile([BL, A], bf16, tag="vab")
            nc.sync.dma_start(out=vab, in_=d_vab[:])
            maskt = cp.tile([BL, L], f32, tag="mask")
            nc.sync.dma_start(out=maskt, in_=d_mask[:])
            zbias = cp.tile([BL, L], f32, tag="zbias")
            nc.sync.dma_start(out=zbias, in_=d_zbias[:])
            padd = cp.tile([BL, 1], f32, tag="padd")
            nc.sync.dma_start(out=padd, in_=d_padd[:])
            lastT = cp.tile([F, BL], f32, tag="lastT")
            nc.sync.dma_start(out=lastT, in_=d_lastT[:])
            w1p = cp.tile([128, 3 * CH], f32, tag="w1p")
            nc.sync.dma_start(out=w1p, in_=d_w1[:])
            w2p = cp.tile([128, 2 * OUT], f32, tag="w2p")
            nc.sync.dma_start(out=w2p, in_=d_w2[:])
            b1r = cp.tile([1, CH], f32, tag="b1r")
            nc.sync.dma_start(out=b1r, in_=d_b1[:])
            b2r = cp.tile([1, OUT], f32, tag="b2r")
            nc.sync.dma_start(out=b2r, in_=d_b2[:])
            eye16 = cp.tile([128, 128], bf16, tag="eye16")
            nc.sync.dma_start(out=eye16, in_=d_eye16[:])
            eye32 = cp.tile([128, 128], f32, tag="eye32")
            nc.sync.dma_start(out=eye32, in_=d_eye32[:])
            ones1 = cp.tile([1, 128], f32, tag="ones1")
            nc.vector.memset(ones1, 1.0)
            onesb = cp.tile([BL, H], bf16, tag="onesb")
            nc.vector.memset(onesb, 1.0)
            if has_gbias:
                gbias = cp.tile([1, 3 * H], f32, tag="gbias")
                nc.sync.dma_start(out=gbias, in_=d_gbias[:])
            if has_obias:
                obias = cp.tile([1, F], f32, tag="obias")
                nc.sync.dma_start(out=obias, in_=d_obias[:])

            scores = cp.tile([BL, L], f32, tag="scores")

            # ---- GRU loop ----
            # Emission order per iteration: G-matmuls for step t first (so the
            # PE queue prioritizes the recurrence), then the deferred
            # attention/output block for step t-1, then gate math for t.
            hT = None
            h_prev = None

            def attn_out_block(t, hT_t):
                # S1 = lastUa + h@Wa ; s_t = tanh(S1).va ; out_t = mask*(h@WoutT)
                s1 = psS1.tile([BL, A], f32, tag="s1", name=f"s1_{t}")
                nc.tensor.matmul(s1, eye16, lastua, start=True, stop=False)
                nc.tensor.matmul(s1, hT_t[:, 0:128], wa[:, 0:A],
                                 start=False, stop=False)
                nc.tensor.matmul(s1, hT_t[:, 128:256], wa[:, A:2 * A],
                                 start=False, stop=True)
                o = psO.tile([BL, F], f32, tag="o", name=f"o_{t}")
                nc.tensor.matmul(o, hT_t[:, 0:128], woutT[:, 0:F],
                                 start=True, stop=False)
                nc.tensor.matmul(o, hT_t[:, 128:256], woutT[:, F:2 * F],
                                 start=False, stop=not has_obias)
                if has_obias:
                    nc.tensor.matmul(o, ones1[:, 0:BL], obias,
                                     start=False, stop=True)
                tanha = wk.tile([BL, A], bf16, tag="tanha", name=f"tanha_{t}")
                nc.scalar.activation(tanha, s1, AF.Tanh)
                scratch = wk.tile([BL, A], bf16, tag="scratch", name=f"scr_{t}")
                nc.vector.scalar_tensor_tensor(
                    scratch, tanha, 1.0, vab, op0=ALU.mult, op1=ALU.mult,
                    accum_out=scores[:, t:t + 1])
                ot = ost.tile([BL, F], f32, tag="ot", name=f"ot_{t}")
                nc.vector.tensor_scalar_mul(ot, o, maskt[:, t:t + 1])
                nc.sync.dma_start(out=d_out[:, t, :], in_=ot)

            for t in range(L):
                xt = xin.tile([F, BL], bf16, tag="xt")
                nc.sync.dma_start(out=xt, in_=d_xT[t])

                grz = psGrz.tile([BL, 2 * H], f32, tag="grz")
                gni = psGni.tile([BL, H], f32, tag="gni")
                if t > 0:
                    gnh = psGnh.tile([BL, H], f32, tag="gnh")
                else:
                    gnh = None

                # x contributions first (no hT dependency; PE can run them early)
                nc.tensor.matmul(grz, xt, wih[:, 0:2 * H], start=True,
                                 stop=(t == 0) and not has_gbias)
                gni_instant = (t == 0) and not has_gbias
                nc.tensor.matmul(gni, xt, wih[:, 2 * H:3 * H],
                                 start=True, stop=gni_instant)
                if has_gbias:
                    nc.tensor.matmul(grz, ones1[:, 0:BL], gbias[:, 0:2 * H],
                                     start=False, stop=(t == 0))
                    nc.tensor.matmul(gni, ones1[:, 0:BL], gbias[:, 2 * H:3 * H],
                                     start=False, stop=(t == 0))
                # h contributions
                if t > 0:
                    nc.tensor.matmul(grz, hT[:, 0:128], whh0[:, 0:2 * H],
                                     start=False, stop=False)
                    nc.tensor.matmul(grz, hT[:, 128:256], whh1[:, 0:2 * H],
                                     start=False, stop=True)
                    nc.tensor.matmul(gnh, hT[:, 0:128], whh0[:, 2 * H:3 * H],
                                     start=True, stop=False)
                    nc.tensor.matmul(gnh, hT[:, 128:256], whh1[:, 2 * H:3 * H],
                                     start=False, stop=True)

                r = wk.tile([BL, H], bf16, tag="r")
                nc.scalar.activation(r, grz[:, 0:H], AF.Sigmoid)
                z = wk.tile([BL, H], bf16, tag="z")
                nc.scalar.activation(z, grz[:, H:2 * H], AF.Sigmoid,
                                     bias=zbias[:, t:t + 1])
                zb = wk.tile([BL, H], bf16, tag="zb")
                nc.vector.scalar_tensor_tensor(zb, z, -1.0, onesb,
                                               op0=ALU.mult, op1=ALU.add)
                hTp = psT.tile([128, 2 * BL], f32, tag="hTp")
                if t > 0:
                    w1t = wk.tile([BL, H], f32, tag="w1t")
                    nc.gpsimd.tensor_mul(w1t, z, h_prev)
                    # transpose w1t into the hT accumulator early (off-chain)
                    nc.tensor.matmul(hTp[:, 0:BL], w1t[:, 0:128], eye32,
                                     is_transpose=True, start=True, stop=False)
                    nc.tensor.matmul(hTp[:, BL:2 * BL], w1t[:, 128:256], eye32,
                                     is_transpose=True, start=False, stop=False)

                n = wk.tile([BL, H], bf16, tag="n")
                if t > 0:
                    u = wk.tile([BL, H], bf16, tag="u")
                    nc.vector.tensor_mul(u, r, gnh)
                    nc.tensor.matmul(gni, eye16, u, start=False, stop=True)
                nc.scalar.activation(n, gni, AF.Tanh)

                w2t = wk.tile([BL, H], f32, tag="w2t")
                nc.vector.tensor_mul(w2t, zb, n)
                # h^T = T(w1t) + T(w2t) accumulated on PE
                nc.tensor.matmul(hTp[:, 0:BL], w2t[:, 0:128], eye32,
                                 is_transpose=True, start=(t == 0), stop=False)
                nc.tensor.matmul(hTp[:, BL:2 * BL], w2t[:, 128:256], eye32,
                                 is_transpose=True, start=False, stop=True)
                hT = wk.tile([128, 2 * BL], bf16, tag="hT")
                nc.vector.tensor_copy(hT[:, 0:BL], hTp[:, 0:BL])
                nc.scalar.copy(hT[:, BL:2 * BL], hTp[:, BL:2 * BL])

                # h natural (off the recurrence chain): hid DMA + next-step w1t
                h = hwork.tile([BL, H], f32, tag="h")
                if t > 0:
                    nc.vector.tensor_add(h, w1t, w2t)
                else:
                    nc.vector.tensor_copy(h, w2t)
                h_prev = h
                nc.sync.dma_start(out=d_hid[t], in_=h)

                # deferred attention/output block for the previous step
                if t > 0:
                    attn_out_block(t - 1, hT_prev_attn)
                hT_prev_attn = hT


            attn_out_block(L - 1, hT)

            # ---- deferred attention context ----
            Et = cp.tile([BL, L], f32, tag="Et")
            nc.scalar.activation(Et, scores, AF.Exp)
            ep = cp.tile([BL, L], f32, tag="ep")
            nc.vector.tensor_mul(ep, Et, maskt)
            d0 = cp.tile([BL, 1], f32, tag="d0")
            nc.vector.tensor_reduce(d0, ep, axis=mybir.AxisListType.X, op=ALU.add)
            dsum = cp.tile([BL, 1], f32, tag="dsum")
            nc.vector.tensor_add(dsum, d0, padd)
            rd = cp.tile([BL, 1], f32, tag="rd")
            nc.vector.reciprocal(rd, dsum)

            acc = psAcc.tile([BL, H], f32, tag="acc")
            for t in range(L):
                hld = hback.tile([BL, H], f32, tag="hld")
                nc.sync.dma_start(out=hld, in_=d_hid[t])
                tmp = wk.tile([BL, H], bf16, tag="tmp")
                nc.vector.tensor_scalar_mul(tmp, hld, ep[:, t:t + 1])
                nc.tensor.matmul(acc, eye16, tmp, start=(t == 0), stop=(t == L - 1))

            ctx = cp.tile([BL, H], f32, tag="ctx")
            nc.vector.tensor_scalar_mul(ctx, acc, rd)

            # ---- cause MLP ----  (PSUM tiles reuse loop tags to stay in 8 banks)
            ctxTp = psS1.tile([128, 2 * BL], f32, tag="s1")
            nc.tensor.transpose(ctxTp[:, 0:BL], ctx[:, 0:128], eye32)
            nc.tensor.transpose(ctxTp[:, BL:2 * BL], ctx[:, 128:256], eye32)
            ctxT = cp.tile([128, 2 * BL], f32, tag="ctxT")
            nc.vector.tensor_copy(ctxT, ctxTp)

            zcp = psGrz.tile([BL, CH], f32, tag="grz")
            nc.tensor.matmul(zcp, ctxT[:, 0:BL], w1p[:, 0:CH], start=True, stop=False)
            nc.tensor.matmul(zcp, ctxT[:, BL:2 * BL], w1p[:, CH:2 * CH],
                             start=False, stop=False)
            nc.tensor.matmul(zcp, lastT, w1p[:, 2 * CH:3 * CH], start=False, stop=False)
            nc.tensor.matmul(zcp, ones1[:, 0:BL], b1r, start=False, stop=True)
            zc = cp.tile([BL, CH], f32, tag="zc")
            nc.scalar.activation(zc, zcp, AF.Relu)

            zcTp = psS1.tile([128, 2 * BL], f32, tag="s1")
            nc.tensor.transpose(zcTp[:, 0:BL], zc[:, 0:128], eye32)
            nc.tensor.transpose(zcTp[:, BL:2 * BL], zc[:, 128:256], eye32)
            zcT = cp.tile([128, 2 * BL], f32, tag="zcT")
            nc.vector.tensor_copy(zcT, zcTp)

            lg = psGrz.tile([BL, OUT], f32, tag="grz")
            nc.tensor.matmul(lg, zcT[:, 0:BL], w2p[:, 0:OUT], start=True, stop=False)
            nc.tensor.matmul(lg, zcT[:, BL:2 * BL], w2p[:, OUT:2 * OUT],
                             start=False, stop=False)
            nc.tensor.matmul(lg, ones1[:, 0:BL], b2r, start=False, stop=True)

            mx = cp.tile([BL, 1], f32, tag="mx")
            nc.vector.tensor_reduce(mx, lg, axis=mybir.AxisListType.X, op=ALU.max)
            nmx = cp.tile([BL, 1], f32, tag="nmx")
            nc.vector.tensor_scalar_mul(nmx, mx, -1.0)
            ex = cp.tile([BL, OUT], f32, tag="ex")
            sm = cp.tile([BL, 1], f32, tag="sm")
            nc.scalar.activation(ex, lg, AF.Exp, bias=nmx, accum_out=sm)
            rs = cp.tile([BL, 1], f32, tag="rs")
            nc.vector.reciprocal(rs, sm)
            fht = cp.tile([BL, OUT], f32, tag="fht")
            nc.vector.tensor_scalar_mul(fht, ex, rs)
            nc.sync.dma_start(out=d_fht[:], in_=fht)

    nc.finalize()
    _CACHE[key] = nc
    return nc


def _prep(input_batch, tte, W_ih, W_hh, b_ih, b_hh, W_out, b_out,
          Wa, Ua, va, W1, b1, W2, b2):
    input_batch = np.asarray(input_batch, np.float32)
    tte = np.asarray(tte, np.int32)
    W_ih = np.asarray(W_ih, np.float32)
    W_hh = np.asarray(W_hh, np.float32)
    b_ih = np.asarray(b_ih, np.float32)
    b_hh = np.asarray(b_hh, np.float32)
    W_out = np.asarray(W_out, np.float32)
    b_out = np.asarray(b_out, np.float32)
    Wa_ = np.asarray(Wa, np.float32)
    Ua = np.asarray(Ua, np.float32)
    va = np.asarray(va, np.float32)
    W1 = np.asarray(W1, np.float32)
    b1 = np.asarray(b1, np.float32)
    W2 = np.asarray(W2, np.float32)
    b2 = np.asarray(b2, np.float32)

    gb = b_ih + b_hh
    has_gbias = bool(np.any(gb))
    has_obias = bool(np.any(b_out))

    # shared (replicated) tensors
    wih_np = np.ascontiguousarray(W_ih.T).astype(BF16)              # [F, 3H]
    WhhT = W_hh.T                                                   # [H, 3H]
    whh_np = np.stack([WhhT[0:128], WhhT[128:256]]).astype(BF16)    # [2,128,3H]
    wa_np = np.concatenate([Wa_[0:128], Wa_[128:256]], axis=1).astype(BF16)
    WoutT = W_out.T                                                 # [H, F]
    woutT_np = np.concatenate([WoutT[0:128], WoutT[128:256]], axis=1).astype(BF16)
    w1_np = np.concatenate([W1[0:128], W1[128:256], W1[256:384]],
                           axis=1).astype(np.float32)               # [128, 3CH]
    w2_np = np.concatenate([W2[0:128], W2[128:256]], axis=1).astype(np.float32)
    b1_np = b1[None, :].astype(np.float32)
    b2_np = b2[None, :].astype(np.float32)
    eye16_np = np.eye(128, dtype=np.float32).astype(BF16)
    eye32_np = np.eye(128, dtype=np.float32)
    vab_np = np.broadcast_to(va, (BL, A)).astype(BF16).copy()
    gb_np = gb[None, :].astype(np.float32)
    ob_np = b_out[None, :].astype(np.float32)

    t_idx = np.arange(L, dtype=np.int32)[None, :]
    in_maps = []
    for c in range(NCORES):
        sl = slice(c * BL, (c + 1) * BL)
        xb = input_batch[sl]                      # [BL, L, F]
        ttec = tte[sl]
        last = xb[np.arange(BL), ttec]            # [BL, F]
        lastUa = last @ Ua                        # [BL, A]
        c_pad = np.tanh(lastUa) @ va              # [BL]
        pad_d = ((L - ttec).astype(np.float32) * np.exp(c_pad)).astype(np.float32)
        active = (t_idx < ttec[:, None]).astype(np.float32)  # [BL, L]
        m = {
            "xT": np.ascontiguousarray(xb.transpose(1, 2, 0)).astype(BF16),
            "wih": wih_np, "whh": whh_np, "wa": wa_np, "woutT": woutT_np,
            "lastua": lastUa.astype(BF16), "vab": vab_np,
            "mask": active, "zbias": (BIG * (1.0 - active)).astype(np.float32),
            "padd": pad_d[:, None],
            "lastT": np.ascontiguousarray(last.T).astype(np.float32),
            "w1p": w1_np, "w2p": w2_np, "b1r": b1_np, "b2r": b2_np,
            "eye16": eye16_np, "eye32": eye32_np,
        }
        if has_gbias:
            m["gbias"] = gb_np
        if has_obias:
            m["obias"] = ob_np
        in_maps.append(m)

    return in_maps, has_gbias, has_obias


def _prep_inputs(inputs):
    """Host-side preprocessing: returns (in_maps, has_gbias, has_obias)."""
    return _prep(**inputs)


def kernel(**inputs):
    in_maps, has_gbias, has_obias = _prep(**inputs)
    nc = _build(has_gbias, has_obias)
    res = run_bass_kernel_spmd(nc, in_maps, core_ids=list(range(NCORES)))
    outs = res.results
    output_batch = np.concatenate([r["out"] for r in outs], axis=0)
    fht = np.concatenate([r["fht"] for r in outs], axis=0)
    return output_batch.astype(np.float32), fht.astype(np.float32)


# revision 38
# speedup vs baseline: 1.0769x; 1.0140x over previous
"""DynamicDeepHit (GRU + additive attention + cause MLP) Bass kernel for 8 trn2 cores.

Sharding: pure data-parallel over batch B=1024 -> 128 per core; weights replicated.

Per-core device program (SPMD, identical on all 8 cores), batch-major layout
[B=128 partitions, features]; L=256 steps fully unrolled:
  - x_t^T loaded pre-transposed from DRAM (host passes [L, F, B] bf16)
  - G = x@W_ihT + h@W_hhT on PE (lhsT = x^T / h^T stationary, f32 PSUM accum);
    x-parts are emitted first so PE runs them while waiting on the recurrence
  - r = sigmoid(Gr); z = sigmoid(Gz + 30*(1-active_t)) — the ragged-sequence
    freeze (h stops updating at t >= tte) is folded into the z gate bias
  - n = tanh(Gni + r*Gnh): r*Gnh on DVE reading PSUM, added into the Gni bank
    via an identity-matmul so tanh reads one PSUM tile
  - h update split as w1 = z*h_old (GPSIMD), w2 = (1-z)*n (DVE); h^T is formed
    by PE transpose-accumulate of w1 and w2 directly in PSUM (keeps the h-add
    off the recurrence critical path); both PSUM->SBUF copies on DVE
  - h (f32) DMAed to a DRAM scratch ring for the deferred attention phase
  - attention score s_t = tanh(h@Wa + last@Ua) . va (PE + ACT tanh + fused
    scalar_tensor_tensor mul-reduce); scores from frozen h are fixed up
    post-loop, so no per-step masking is needed
  - out_t = active_t * (h@W_outT) -> DMA (masking via per-partition scalar)
Post-loop: E = exp(scores); e' = E*mask; d = sum(e') + pad_d where pad_d is the
host-computed softmax mass of the padded positions ((L-tte)*exp(tanh(lastUa).va));
context = (sum_t e'_t * h_t)/d accumulated via PE identity-matmuls; then the
cause MLP + softmax -> fht.

Host side precomputes: transposed/packed weights (bf16 for matmuls), last
(gather at tte), lastUa, active mask, z-freeze bias, pad_d, identities.

PSUM budget (8 banks, one accumulation group per bank at a time):
  grz x2 (double-buffered), gni, gnh, s1, o, hTp, acc.
"""

import numpy as np
import sys

for _p in ("/opt/trn_rl_repo", "/root/.axon_site/_ro/trn_rl_repo"):
    if _p not in sys.path:
        sys.path.insert(0, _p)

import ml_dtypes

import concourse.bacc as bacc
import concourse.bass as bass
import concourse.tile as tile
from concourse import mybir
from concourse.bass_utils import run_bass_kernel_spmd

BF16 = ml_dtypes.bfloat16

# Problem dims (hardcoded per spec)
B, L, F, H = 1024, 256, 128, 256
A, CH, OUT = 128, 256, 512
NCORES = 8
BL = B // NCORES  # 128 per core
BIG = 30.0

f32 = mybir.dt.float32
bf16 = mybir.dt.bfloat16
AF = mybir.ActivationFunctionType
ALU = mybir.AluOpType

_CACHE = {}


def _build(has_gbias: bool, has_obias: bool):
    key = (has_gbias, has_obias)
    if key in _CACHE:
        return _CACHE[key]

    nc = bacc.Bacc("TRN2", target_bir_lowering=False, debug=False,
                   num_devices=NCORES)

    # ---- DRAM I/O (per-core shapes) ----
    d_xT = nc.dram_tensor("xT", [L, F, BL], bf16, kind="ExternalInput")
    d_wih = nc.dram_tensor("wih", [F, 3 * H], bf16, kind="ExternalInput")
    d_whh = nc.dram_tensor("whh", [2, 128, 3 * H], bf16, kind="ExternalInput")
    d_wa = nc.dram_tensor("wa", [128, 2 * A], bf16, kind="ExternalInput")
    d_woutT = nc.dram_tensor("woutT", [128, 2 * F], bf16, kind="ExternalInput")
    d_lastua = nc.dram_tensor("lastua", [BL, A], bf16, kind="ExternalInput")
    d_vab = nc.dram_tensor("vab", [BL, A], bf16, kind="ExternalInput")
    d_mask = nc.dram_tensor("mask", [BL, L], f32, kind="ExternalInput")
    d_zbias = nc.dram_tensor("zbias", [BL, L], f32, kind="ExternalInput")
    d_padd = nc.dram_tensor("padd", [BL, 1], f32, kind="ExternalInput")
    d_lastT = nc.dram_tensor("lastT", [F, BL], f32, kind="ExternalInput")
    d_w1 = nc.dram_tensor("w1p", [128, 3 * CH], f32, kind="ExternalInput")
    d_w2 = nc.dram_tensor("w2p", [128, 2 * OUT], f32, kind="ExternalInput")
    d_b1 = nc.dram_tensor("b1r", [1, CH], f32, kind="ExternalInput")
    d_b2 = nc.dram_tensor("b2r", [1, OUT], f32, kind="ExternalInput")
    d_eye16 = nc.dram_tensor("eye16", [128, 128], bf16, kind="ExternalInput")
    d_eye32 = nc.dram_tensor("eye32", [128, 128], f32, kind="ExternalInput")
    if has_gbias:
        d_gbias = nc.dram_tensor("gbias", [1, 3 * H], f32, kind="ExternalInput")
    if has_obias:
        d_obias = nc.dram_tensor("obias", [1, F], f32, kind="ExternalInput")

    d_hid = nc.dram_tensor("hid", [L, BL, H], f32)
    d_out = nc.dram_tensor("out", [BL, L, F], f32, kind="ExternalOutput")
    d_fht = nc.dram_tensor("fht", [BL, OUT], f32, kind="ExternalOutput")

    with tile.TileContext(nc) as tc:
        with (
            tc.tile_pool(name="const", bufs=1) as cp,
            tc.tile_pool(name="hwork", bufs=4) as hwork,
            tc.tile_pool(name="hback", bufs=8) as hback,
            tc.tile_pool(name="work", bufs=4) as wk,
            tc.tile_pool(name="xin", bufs=6) as xin,
            tc.tile_pool(name="ostage", bufs=6) as ost,
            tc.tile_pool(name="psGrz", bufs=2, space="PSUM") as psGrz,
            tc.tile_pool(name="psGni", bufs=1, space="PSUM") as psGni,
            tc.tile_pool(name="psGnh", bufs=1, space="PSUM") as psGnh,
            tc.tile_pool(name="psS1", bufs=1, space="PSUM") as psS1,
            tc.tile_pool(name="psO", bufs=1, space="PSUM") as psO,
            tc.tile_pool(name="psT", bufs=1, space="PSUM") as psT,
            tc.tile_pool(name="psAcc", bufs=1, space="PSUM") as psAcc,
        ):
            # ---- constants into SBUF ----
            wih = cp.tile([F, 3 * H], bf16, tag="wih")
            nc.sync.dma_start(out=wih, in_=d_wih[:])
            whh0 = cp.tile([128, 3 * H], bf16, tag="whh0")
            nc.sync.dma_start(out=whh0, in_=d_whh[0])
            whh1 = cp.tile([128, 3 * H], bf16, tag="whh1")
            nc.sync.dma_start(out=whh1, in_=d_whh[1])
            wa = cp.tile([128, 2 * A], bf16, tag="wa")
            nc.sync.dma_start(out=wa, in_=d_wa[:])
            woutT = cp.tile([128, 2 * F], bf16, tag="woutT")
            nc.sync.dma_start(out=woutT, in_=d_woutT[:])
            lastua = cp.tile([BL, A], bf16, tag="lastua")
            nc.sync.dma_start(out=lastua, in_=d_lastua[:])
            vab = cp.tile([BL, A], bf16, tag="vab")
            nc.sync.dma_start(out=vab, in_=d_vab[:])
            maskt = cp.tile([BL, L], f32, tag="mask")
            nc.sync.dma_start(out=maskt, in_=d_mask[:])
            zbias = cp.tile([BL, L], f32, tag="zbias")
            nc.sync.dma_start(out=zbias, in_=d_zbias[:])
            padd = cp.tile([BL, 1], f32, tag="padd")
            nc.sync.dma_start(out=padd, in_=d_padd[:])
            lastT = cp.tile([F, BL], f32, tag="lastT")
            nc.sync.dma_start(out=lastT, in_=d_lastT[:])
            w1p = cp.tile([128, 3 * CH], f32, tag="w1p")
            nc.sync.dma_start(out=w1p, in_=d_w1[:])
            w2p = cp.tile([128, 2 * OUT], f32, tag="w2p")
            nc.sync.dma_start(out=w2p, in_=d_w2[:])
            b1r = cp.tile([1, CH], f32, tag="b1r")
            nc.sync.dma_start(out=b1r, in_=d_b1[:])
            b2r = cp.tile([1, OUT], f32, tag="b2r")
            nc.sync.dma_start(out=b2r, in_=d_b2[:])
            eye16 = cp.tile([128, 128], bf16, tag="eye16")
            nc.sync.dma_start(out=eye16, in_=d_eye16[:])
            eye32 = cp.tile([128, 128], f32, tag="eye32")
            nc.sync.dma_start(out=eye32, in_=d_eye32[:])
            ones1 = cp.tile([1, 128], f32, tag="ones1")
            nc.vector.memset(ones1, 1.0)
            onesb = cp.tile([BL, H], bf16, tag="onesb")
            nc.vector.memset(onesb, 1.0)
            if has_gbias:
                gbias = cp.tile([1, 3 * H], f32, tag="gbias")
                nc.sync.dma_start(out=gbias, in_=d_gbias[:])
            if has_obias:
                obias = cp.tile([1, F], f32, tag="obias")
                nc.sync.dma_start(out=obias, in_=d_obias[:])

            scores = cp.tile([BL, L], f32, tag="scores")

            # ---- GRU loop ----
            # Emission order per iteration: G-matmuls for step t first (so the
            # PE queue prioritizes the recurrence), then the deferred
            # attention/output block for step t-1, then gate math for t.
            hT = None
            h_prev = None

            def attn_out_block(t, hT_t):
                # S1 = lastUa + h@Wa ; s_t = tanh(S1).va ; out_t = mask*(h@WoutT)
                s1 = psS1.tile([BL, A], f32, tag="s1", name=f"s1_{t}")
                nc.tensor.matmul(s1, eye16, lastua, start=True, stop=False)
                nc.tensor.matmul(s1, hT_t[:, 0:128], wa[:, 0:A],
                                 start=False, stop=False)
                nc.tensor.matmul(s1, hT_t[:, 128:256], wa[:, A:2 * A],
                                 start=False, stop=True)
                o = psO.tile([BL, F], f32, tag="o", name=f"o_{t}")
                nc.tensor.matmul(o, hT_t[:, 0:128], woutT[:, 0:F],
                                 start=True, stop=False)
                nc.tensor.matmul(o, hT_t[:, 128:256], woutT[:, F:2 * F],
                                 start=False, stop=not has_obias)
                if has_obias:
                    nc.tensor.matmul(o, ones1[:, 0:BL], obias,
                                     start=False, stop=True)
                tanha = wk.tile([BL, A], bf16, tag="tanha", name=f"tanha_{t}")
                nc.scalar.activation(tanha, s1, AF.Tanh)
                scratch = wk.tile([BL, A], bf16, tag="scratch", name=f"scr_{t}")
                nc.vector.scalar_tensor_tensor(
                    scratch, tanha, 1.0, vab, op0=ALU.mult, op1=ALU.mult,
                    accum_out=scores[:, t:t + 1])
                ot = ost.tile([BL, F], f32, tag="ot", name=f"ot_{t}")
                nc.vector.tensor_scalar_mul(ot, o, maskt[:, t:t + 1])
                nc.sync.dma_start(out=d_out[:, t, :], in_=ot)

            for t in range(L):
                xt = xin.tile([F, BL], bf16, tag="xt")
                nc.sync.dma_start(out=xt, in_=d_xT[t])

                grz = psGrz.tile([BL, 2 * H], f32, tag="grz")
                gni = psGni.tile([BL, H], f32, tag="gni")
                if t > 0:
                    gnh = psGnh.tile([BL, H], f32, tag="gnh")
                else:
                    gnh = None

                # x contributions first (no hT dependency; PE can run them early)
                nc.tensor.matmul(grz, xt, wih[:, 0:2 * H], start=True,
                                 stop=(t == 0) and not has_gbias)
                gni_instant = (t == 0) and not has_gbias
                nc.tensor.matmul(gni, xt, wih[:, 2 * H:3 * H],
                                 start=True, stop=gni_instant)
                if has_gbias:
                    nc.tensor.matmul(grz, ones1[:, 0:BL], gbias[:, 0:2 * H],
                                     start=False, stop=(t == 0))
                    nc.tensor.matmul(gni, ones1[:, 0:BL], gbias[:, 2 * H:3 * H],
                                     start=False, stop=(t == 0))
                # h contributions
                if t > 0:
                    nc.tensor.matmul(grz, hT[:, 0:128], whh0[:, 0:2 * H],
                                     start=False, stop=False)
                    nc.tensor.matmul(grz, hT[:, 128:256], whh1[:, 0:2 * H],
                                     start=False, stop=True)
                    nc.tensor.matmul(gnh, hT[:, 0:128], whh0[:, 2 * H:3 * H],
                                     start=True, stop=False)
                    nc.tensor.matmul(gnh, hT[:, 128:256], whh1[:, 2 * H:3 * H],
                                     start=False, stop=True)

                r = wk.tile([BL, H], bf16, tag="r")
                nc.scalar.activation(r, grz[:, 0:H], AF.Sigmoid)
                z = wk.tile([BL, H], bf16, tag="z")
                nc.scalar.activation(z, grz[:, H:2 * H], AF.Sigmoid,
                                     bias=zbias[:, t:t + 1])
                zb = wk.tile([BL, H], bf16, tag="zb")
                nc.vector.scalar_tensor_tensor(zb, z, -1.0, onesb,
                                               op0=ALU.mult, op1=ALU.add)
                hTp = psT.tile([128, 2 * BL], f32, tag="hTp")
                if t > 0:
                    w1t = wk.tile([BL, H], f32, tag="w1t")
                    nc.gpsimd.tensor_mul(w1t, z, h_prev)
                    # transpose w1t into the hT accumulator early (off-chain)
                    nc.tensor.matmul(hTp[:, 0:BL], w1t[:, 0:128], eye32,
                                     is_transpose=True, start=True, stop=False)
                    nc.tensor.matmul(hTp[:, BL:2 * BL], w1t[:, 128:256], eye32,
                                     is_transpose=True, start=False, stop=False)

                n = wk.tile([BL, H], bf16, tag="n")
                if t > 0:
                    u = wk.tile([BL, H], bf16, tag="u")
                    nc.vector.tensor_mul(u, r, gnh)
                    nc.tensor.matmul(gni, eye16, u, start=False, stop=True)
                nc.scalar.activation(n, gni, AF.Tanh)

                w2t = wk.tile([BL, H], f32, tag="w2t")
                nc.vector.tensor_mul(w2t, zb, n)
                # h^T = T(w1t) + T(w2t) accumulated on PE
                nc.tensor.matmul(hTp[:, 0:BL], w2t[:, 0:128], eye32,
                                 is_transpose=True, start=(t == 0), stop=False)
                nc.tensor.matmul(hTp[:, BL:2 * BL], w2t[:, 128:256], eye32,
                                 is_transpose=True, start=False, stop=True)
                hT = wk.tile([128, 2 * BL], bf16, tag="hT")
                nc.vector.tensor_copy(hT[:, 0:BL], hTp[:, 0:BL])
                nc.vector.tensor_copy(hT[:, BL:2 * BL], hTp[:, BL:2 * BL])

                # h natural (off the recurrence chain): hid DMA + next-step w1t
                h = hwork.tile([BL, H], f32, tag="h")
                if t > 0:
                    nc.vector.tensor_add(h, w1t, w2t)
                else:
                    nc.vector.tensor_copy(h, w2t)
                h_prev = h
                nc.sync.dma_start(out=d_hid[t], in_=h)

                # deferred attention/output block for the previous step
                if t > 0:
                    attn_out_block(t - 1, hT_prev_attn)
                hT_prev_attn = hT


            attn_out_block(L - 1, hT)

            # ---- deferred attention context ----
            Et = cp.tile([BL, L], f32, tag="Et")
            nc.scalar.activation(Et, scores, AF.Exp)
            ep = cp.tile([BL, L], f32, tag="ep")
            nc.vector.tensor_mul(ep, Et, maskt)
            d0 = cp.tile([BL, 1], f32, tag="d0")
            nc.vector.tensor_reduce(d0, ep, axis=mybir.AxisListType.X, op=ALU.add)
            dsum = cp.tile([BL, 1], f32, tag="dsum")
            nc.vector.tensor_add(dsum, d0, padd)
            rd = cp.tile([BL, 1], f32, tag="rd")
            nc.vector.reciprocal(rd, dsum)

            acc = psAcc.tile([BL, H], f32, tag="acc")
            for t in range(L):
                hld = hback.tile([BL, H], f32, tag="hld")
                nc.sync.dma_start(out=hld, in_=d_hid[t])
                tmp = wk.tile([BL, H], bf16, tag="tmp")
                nc.vector.tensor_scalar_mul(tmp, hld, ep[:, t:t + 1])
                nc.tensor.matmul(acc, eye16, tmp, start=(t == 0), stop=(t == L - 1))

            ctx = cp.tile([BL, H], f32, tag="ctx")
            nc.vector.tensor_scalar_mul(ctx, acc, rd)

            # ---- cause MLP ----  (PSUM tiles reuse loop tags to stay in 8 banks)
            ctxTp = psS1.tile([128, 2 * BL], f32, tag="s1")
            nc.tensor.transpose(ctxTp[:, 0:BL], ctx[:, 0:128], eye32)
            nc.tensor.transpose(ctxTp[:, BL:2 * BL], ctx[:, 128:256], eye32)
            ctxT = cp.tile([128, 2 * BL], f32, tag="ctxT")
            nc.vector.tensor_copy(ctxT, ctxTp)

            zcp = psGrz.tile([BL, CH], f32, tag="grz")
            nc.tensor.matmul(zcp, ctxT[:, 0:BL], w1p[:, 0:CH], start=True, stop=False)
            nc.tensor.matmul(zcp, ctxT[:, BL:2 * BL], w1p[:, CH:2 * CH],
                             start=False, stop=False)
            nc.tensor.matmul(zcp, lastT, w1p[:, 2 * CH:3 * CH], start=False, stop=False)
            nc.tensor.matmul(zcp, ones1[:, 0:BL], b1r, start=False, stop=True)
            zc = cp.tile([BL, CH], f32, tag="zc")
            nc.scalar.activation(zc, zcp, AF.Relu)

            zcTp = psS1.tile([128, 2 * BL], f32, tag="s1")
            nc.tensor.transpose(zcTp[:, 0:BL], zc[:, 0:128], eye32)
            nc.tensor.transpose(zcTp[:, BL:2 * BL], zc[:, 128:256], eye32)
            zcT = cp.tile([128, 2 * BL], f32, tag="zcT")
            nc.vector.tensor_copy(zcT, zcTp)

            lg = psGrz.tile([BL, OUT], f32, tag="grz")
            nc.tensor.matmul(lg, zcT[:, 0:BL], w2p[:, 0:OUT], start=True, stop=False)
            nc.tensor.matmul(lg, zcT[:, BL:2 * BL], w2p[:, OUT:2 * OUT],
                             start=False, stop=False)
            nc.tensor.matmul(lg, ones1[:, 0:BL], b2r, start=False, stop=True)

            mx = cp.tile([BL, 1], f32, tag="mx")
            nc.vector.tensor_reduce(mx, lg, axis=mybir.AxisListType.X, op=ALU.max)
            nmx = cp.tile([BL, 1], f32, tag="nmx")
            nc.vector.tensor_scalar_mul(nmx, mx, -1.0)
            ex = cp.tile([BL, OUT], f32, tag="ex")
            sm = cp.tile([BL, 1], f32, tag="sm")
            nc.scalar.activation(ex, lg, AF.Exp, bias=nmx, accum_out=sm)
            rs = cp.tile([BL, 1], f32, tag="rs")
            nc.vector.reciprocal(rs, sm)
            fht = cp.tile([BL, OUT], f32, tag="fht")
            nc.vector.tensor_scalar_mul(fht, ex, rs)
            nc.sync.dma_start(out=d_fht[:], in_=fht)

    nc.finalize()
    _CACHE[key] = nc
    return nc


def _prep(input_batch, tte, W_ih, W_hh, b_ih, b_hh, W_out, b_out,
          Wa, Ua, va, W1, b1, W2, b2):
    input_batch = np.asarray(input_batch, np.float32)
    tte = np.asarray(tte, np.int32)
    W_ih = np.asarray(W_ih, np.float32)
    W_hh = np.asarray(W_hh, np.float32)
    b_ih = np.asarray(b_ih, np.float32)
    b_hh = np.asarray(b_hh, np.float32)
    W_out = np.asarray(W_out, np.float32)
    b_out = np.asarray(b_out, np.float32)
    Wa_ = np.asarray(Wa, np.float32)
    Ua = np.asarray(Ua, np.float32)
    va = np.asarray(va, np.float32)
    W1 = np.asarray(W1, np.float32)
    b1 = np.asarray(b1, np.float32)
    W2 = np.asarray(W2, np.float32)
    b2 = np.asarray(b2, np.float32)

    gb = b_ih + b_hh
    has_gbias = bool(np.any(gb))
    has_obias = bool(np.any(b_out))

    # shared (replicated) tensors
    wih_np = np.ascontiguousarray(W_ih.T).astype(BF16)              # [F, 3H]
    WhhT = W_hh.T                                                   # [H, 3H]
    whh_np = np.stack([WhhT[0:128], WhhT[128:256]]).astype(BF16)    # [2,128,3H]
    wa_np = np.concatenate([Wa_[0:128], Wa_[128:256]], axis=1).astype(BF16)
    WoutT = W_out.T                                                 # [H, F]
    woutT_np = np.concatenate([WoutT[0:128], WoutT[128:256]], axis=1).astype(BF16)
    w1_np = np.concatenate([W1[0:128], W1[128:256], W1[256:384]],
                           axis=1).astype(np.float32)               # [128, 3CH]
    w2_np = np.concatenate([W2[0:128], W2[128:256]], axis=1).astype(np.float32)
    b1_np = b1[None, :].astype(np.float32)
    b2_np = b2[None, :].astype(np.float32)
    eye16_np = np.eye(128, dtype=np.float32).astype(BF16)
    eye32_np = np.eye(128, dtype=np.float32)
    vab_np = np.broadcast_to(va, (BL, A)).astype(BF16).copy()
    gb_np = gb[None, :].astype(np.float32)
    ob_np = b_out[None, :].astype(np.float32)

    t_idx = np.arange(L, dtype=np.int32)[None, :]
    in_maps = []
    for c in range(NCORES):
        sl = slice(c * BL, (c + 1) * BL)
        xb = input_batch[sl]                      # [BL, L, F]
        ttec = tte[sl]
        last = xb[np.arange(BL), ttec]            # [BL, F]
        lastUa = last @ Ua                        # [BL, A]
        c_pad = np.tanh(lastUa) @ va              # [BL]
        pad_d = ((L - ttec).astype(np.float32) * np.exp(c_pad)).astype(np.float32)
        active = (t_idx < ttec[:, None]).astype(np.float32)  # [BL, L]
        m = {
            "xT": np.ascontiguousarray(xb.transpose(1, 2, 0)).astype(BF16),
            "wih": wih_np, "whh": whh_np, "wa": wa_np, "woutT": woutT_np,
            "lastua": lastUa.astype(BF16), "vab": vab_np,
            "mask": active, "zbias": (BIG * (1.0 - active)).astype(np.float32),
            "padd": pad_d[:, None],
            "lastT": np.ascontiguousarray(last.T).astype(np.float32),
            "w1p": w1_np, "w2p": w2_np, "b1r": b1_np, "b2r": b2_np,
            "eye16": eye16_np, "eye32": eye32_np,
        }
        if has_gbias:
            m["gbias"] = gb_np
        if has_obias:
            m["obias"] = ob_np
        in_maps.append(m)

    return in_maps, has_gbias, has_obias


def _prep_inputs(inputs):
    """Host-side preprocessing: returns (in_maps, has_gbias, has_obias)."""
    return _prep(**inputs)


def kernel(**inputs):
    in_maps, has_gbias, has_obias = _prep(**inputs)
    nc = _build(has_gbias, has_obias)
    res = run_bass_kernel_spmd(nc, in_maps, core_ids=list(range(NCORES)))
    outs = res.results
    output_batch = np.concatenate([r["out"] for r in outs], axis=0)
    fht = np.concatenate([r["fht"] for r in outs], axis=0)
    return output_batch.astype(np.float32), fht.astype(np.float32)


# revision 40
# speedup vs baseline: 1.1714x; 1.0878x over previous
"""DynamicDeepHit (GRU + additive attention + cause MLP) Bass kernel for 8 trn2 cores.

Sharding: pure data-parallel over batch B=1024 -> 128 per core; weights replicated.

Per-core device program (SPMD, identical on all 8 cores), batch-major layout
[B=128 partitions, features]; L=256 steps fully unrolled:
  - x_t^T loaded pre-transposed from DRAM (host passes [L, F, B] bf16)
  - G = x@W_ihT + h@W_hhT on PE (lhsT = x^T / h^T stationary, f32 PSUM accum);
    x-parts are emitted first so PE runs them while waiting on the recurrence
  - r = sigmoid(Gr); z = sigmoid(Gz + 30*(1-active_t)) — the ragged-sequence
    freeze (h stops updating at t >= tte) is folded into the z gate bias
  - n = tanh(Gni + r*Gnh): r*Gnh on DVE reading PSUM, added into the Gni bank
    via an identity-matmul so tanh reads one PSUM tile
  - h update split as w1 = z*h_old (GPSIMD), w2 = (1-z)*n (DVE); h^T is formed
    by PE transpose-accumulate of w1 and w2 directly in PSUM (keeps the h-add
    off the recurrence critical path); both PSUM->SBUF copies on DVE
  - h kept in a 256-deep bf16 SBUF ring for the deferred attention phase
  - attention score s_t = tanh(h@Wa + last@Ua) . va (PE + ACT tanh + fused
    scalar_tensor_tensor mul-reduce); scores from frozen h are fixed up
    post-loop, so no per-step masking is needed
  - out_t = active_t * (h@W_outT) -> DMA (masking via per-partition scalar)
Post-loop: E = exp(scores); e' = E*mask; d = sum(e') + pad_d where pad_d is the
host-computed softmax mass of the padded positions ((L-tte)*exp(tanh(lastUa).va));
context = (sum_t e'_t * h_t)/d accumulated via PE identity-matmuls; then the
cause MLP + softmax -> fht.

Host side precomputes: transposed/packed weights (bf16 for matmuls), last
(gather at tte), lastUa, active mask, z-freeze bias, pad_d, identities.

PSUM budget (8 banks, one accumulation group per bank at a time):
  grz x2 (double-buffered), gni, gnh, s1, o, hTp, acc.
"""

import numpy as np
import sys

for _p in ("/opt/trn_rl_repo", "/root/.axon_site/_ro/trn_rl_repo"):
    if _p not in sys.path:
        sys.path.insert(0, _p)

import ml_dtypes

import concourse.bacc as bacc
import concourse.bass as bass
import concourse.tile as tile
from concourse import mybir
from concourse.bass_utils import run_bass_kernel_spmd

BF16 = ml_dtypes.bfloat16

# Problem dims (hardcoded per spec)
B, L, F, H = 1024, 256, 128, 256
A, CH, OUT = 128, 256, 512
NCORES = 8
BL = B // NCORES  # 128 per core
BIG = 30.0

f32 = mybir.dt.float32
bf16 = mybir.dt.bfloat16
AF = mybir.ActivationFunctionType
ALU = mybir.AluOpType

_CACHE = {}


def _build(has_gbias: bool, has_obias: bool):
    key = (has_gbias, has_obias)
    if key in _CACHE:
        return _CACHE[key]

    nc = bacc.Bacc("TRN2", target_bir_lowering=False, debug=False,
                   num_devices=NCORES)

    # ---- DRAM I/O (per-core shapes) ----
    d_xT = nc.dram_tensor("xT", [L, F, BL], bf16, kind="ExternalInput")
    d_wih = nc.dram_tensor("wih", [F, 3 * H], bf16, kind="ExternalInput")
    d_whh = nc.dram_tensor("whh", [2, 128, 3 * H], bf16, kind="ExternalInput")
    d_wa = nc.dram_tensor("wa", [128, 2 * A], bf16, kind="ExternalInput")
    d_woutT = nc.dram_tensor("woutT", [128, 2 * F], bf16, kind="ExternalInput")
    d_lastua = nc.dram_tensor("lastua", [BL, A], bf16, kind="ExternalInput")
    d_vab = nc.dram_tensor("vab", [BL, A], bf16, kind="ExternalInput")
    d_mask = nc.dram_tensor("mask", [BL, L], f32, kind="ExternalInput")
    d_zbias = nc.dram_tensor("zbias", [BL, L], f32, kind="ExternalInput")
    d_padd = nc.dram_tensor("padd", [BL, 1], f32, kind="ExternalInput")
    d_lastT = nc.dram_tensor("lastT", [F, BL], f32, kind="ExternalInput")
    d_w1 = nc.dram_tensor("w1p", [128, 3 * CH], f32, kind="ExternalInput")
    d_w2 = nc.dram_tensor("w2p", [128, 2 * OUT], f32, kind="ExternalInput")
    d_b1 = nc.dram_tensor("b1r", [1, CH], f32, kind="ExternalInput")
    d_b2 = nc.dram_tensor("b2r", [1, OUT], f32, kind="ExternalInput")
    d_eye16 = nc.dram_tensor("eye16", [128, 128], bf16, kind="ExternalInput")
    d_eye32 = nc.dram_tensor("eye32", [128, 128], f32, kind="ExternalInput")
    if has_gbias:
        d_gbias = nc.dram_tensor("gbias", [1, 3 * H], f32, kind="ExternalInput")
    if has_obias:
        d_obias = nc.dram_tensor("obias", [1, F], f32, kind="ExternalInput")

    d_out = nc.dram_tensor("out", [BL, L, F], f32, kind="ExternalOutput")
    d_fht = nc.dram_tensor("fht", [BL, OUT], f32, kind="ExternalOutput")

    with tile.TileContext(nc) as tc:
        with (
            tc.tile_pool(name="const", bufs=1) as cp,
            tc.tile_pool(name="hwork", bufs=4) as hwork,
            tc.tile_pool(name="hring", bufs=L + 2) as hring,
            tc.tile_pool(name="work", bufs=4) as wk,
            tc.tile_pool(name="xin", bufs=6) as xin,
            tc.tile_pool(name="ostage", bufs=6) as ost,
            tc.tile_pool(name="psGrz", bufs=2, space="PSUM") as psGrz,
            tc.tile_pool(name="psGni", bufs=1, space="PSUM") as psGni,
            tc.tile_pool(name="psGnh", bufs=1, space="PSUM") as psGnh,
            tc.tile_pool(name="psS1", bufs=1, space="PSUM") as psS1,
            tc.tile_pool(name="psO", bufs=1, space="PSUM") as psO,
            tc.tile_pool(name="psT", bufs=1, space="PSUM") as psT,
            tc.tile_pool(name="psAcc", bufs=1, space="PSUM") as psAcc,
        ):
            # ---- constants into SBUF ----
            wih = cp.tile([F, 3 * H], bf16, tag="wih")
            nc.sync.dma_start(out=wih, in_=d_wih[:])
            whh0 = cp.tile([128, 3 * H], bf16, tag="whh0")
            nc.sync.dma_start(out=whh0, in_=d_whh[0])
            whh1 = cp.tile([128, 3 * H], bf16, tag="whh1")
            nc.sync.dma_start(out=whh1, in_=d_whh[1])
            wa = cp.tile([128, 2 * A], bf16, tag="wa")
            nc.sync.dma_start(out=wa, in_=d_wa[:])
            woutT = cp.tile([128, 2 * F], bf16, tag="woutT")
            nc.sync.dma_start(out=woutT, in_=d_woutT[:])
            lastua = cp.tile([BL, A], bf16, tag="lastua")
            nc.sync.dma_start(out=lastua, in_=d_lastua[:])
            vab = cp.tile([BL, A], bf16, tag="vab")
            nc.sync.dma_start(out=vab, in_=d_vab[:])
            maskt = cp.tile([BL, L], f32, tag="mask")
            nc.sync.dma_start(out=maskt, in_=d_mask[:])
            zbias = cp.tile([BL, L], f32, tag="zbias")
            nc.sync.dma_start(out=zbias, in_=d_zbias[:])
            padd = cp.tile([BL, 1], f32, tag="padd")
            nc.sync.dma_start(out=padd, in_=d_padd[:])
            lastT = cp.tile([F, BL], f32, tag="lastT")
            nc.sync.dma_start(out=lastT, in_=d_lastT[:])
            w1p = cp.tile([128, 3 * CH], f32, tag="w1p")
            nc.sync.dma_start(out=w1p, in_=d_w1[:])
            w2p = cp.tile([128, 2 * OUT], f32, tag="w2p")
            nc.sync.dma_start(out=w2p, in_=d_w2[:])
            b1r = cp.tile([1, CH], f32, tag="b1r")
            nc.sync.dma_start(out=b1r, in_=d_b1[:])
            b2r = cp.tile([1, OUT], f32, tag="b2r")
            nc.sync.dma_start(out=b2r, in_=d_b2[:])
            eye16 = cp.tile([128, 128], bf16, tag="eye16")
            nc.sync.dma_start(out=eye16, in_=d_eye16[:])
            eye32 = cp.tile([128, 128], f32, tag="eye32")
            nc.sync.dma_start(out=eye32, in_=d_eye32[:])
            ones1 = cp.tile([1, 128], f32, tag="ones1")
            nc.vector.memset(ones1, 1.0)
            onesb = cp.tile([BL, H], bf16, tag="onesb")
            nc.vector.memset(onesb, 1.0)
            if has_gbias:
                gbias = cp.tile([1, 3 * H], f32, tag="gbias")
                nc.sync.dma_start(out=gbias, in_=d_gbias[:])
            if has_obias:
                obias = cp.tile([1, F], f32, tag="obias")
                nc.sync.dma_start(out=obias, in_=d_obias[:])

            scores = cp.tile([BL, L], f32, tag="scores")

            # ---- GRU loop ----
            # Emission order per iteration: G-matmuls for step t first (so the
            # PE queue prioritizes the recurrence), then the deferred
            # attention/output block for step t-1, then gate math for t.
            hT = None
            h_prev = None
            h_hist = []

            def attn_out_block(t, hT_t):
                # S1 = lastUa + h@Wa ; s_t = tanh(S1).va ; out_t = mask*(h@WoutT)
                s1 = psS1.tile([BL, A], f32, tag="s1", name=f"s1_{t}")
                nc.tensor.matmul(s1, eye16, lastua, start=True, stop=False)
                nc.tensor.matmul(s1, hT_t[:, 0:128], wa[:, 0:A],
                                 start=False, stop=False)
                nc.tensor.matmul(s1, hT_t[:, 128:256], wa[:, A:2 * A],
                                 start=False, stop=True)
                o = psO.tile([BL, F], f32, tag="o", name=f"o_{t}")
                nc.tensor.matmul(o, hT_t[:, 0:128], woutT[:, 0:F],
                                 start=True, stop=False)
                nc.tensor.matmul(o, hT_t[:, 128:256], woutT[:, F:2 * F],
                                 start=False, stop=not has_obias)
                if has_obias:
                    nc.tensor.matmul(o, ones1[:, 0:BL], obias,
                                     start=False, stop=True)
                tanha = wk.tile([BL, A], bf16, tag="tanha", name=f"tanha_{t}")
                nc.scalar.activation(tanha, s1, AF.Tanh)
                scratch = wk.tile([BL, A], bf16, tag="scratch", name=f"scr_{t}")
                nc.vector.scalar_tensor_tensor(
                    scratch, tanha, 1.0, vab, op0=ALU.mult, op1=ALU.mult,
                    accum_out=scores[:, t:t + 1])
                ot = ost.tile([BL, F], f32, tag="ot", name=f"ot_{t}")
                nc.vector.tensor_scalar_mul(ot, o, maskt[:, t:t + 1])
                nc.sync.dma_start(out=d_out[:, t, :], in_=ot)

            for t in range(L):
                xt = xin.tile([F, BL], bf16, tag="xt")
                nc.sync.dma_start(out=xt, in_=d_xT[t])

                grz = psGrz.tile([BL, 2 * H], f32, tag="grz")
                gni = psGni.tile([BL, H], f32, tag="gni")
                if t > 0:
                    gnh = psGnh.tile([BL, H], f32, tag="gnh")
                else:
                    gnh = None

                # x contributions first (no hT dependency; PE can run them early)
                nc.tensor.matmul(grz, xt, wih[:, 0:2 * H], start=True,
                                 stop=(t == 0) and not has_gbias)
                gni_instant = (t == 0) and not has_gbias
                nc.tensor.matmul(gni, xt, wih[:, 2 * H:3 * H],
                                 start=True, stop=gni_instant)
                if has_gbias:
                    nc.tensor.matmul(grz, ones1[:, 0:BL], gbias[:, 0:2 * H],
                                     start=False, stop=(t == 0))
                    nc.tensor.matmul(gni, ones1[:, 0:BL], gbias[:, 2 * H:3 * H],
                                     start=False, stop=(t == 0))
                # h contributions
                if t > 0:
                    nc.tensor.matmul(grz, hT[:, 0:128], whh0[:, 0:2 * H],
                                     start=False, stop=False)
                    nc.tensor.matmul(grz, hT[:, 128:256], whh1[:, 0:2 * H],
                                     start=False, stop=True)
                    nc.tensor.matmul(gnh, hT[:, 0:128], whh0[:, 2 * H:3 * H],
                                     start=True, stop=False)
                    nc.tensor.matmul(gnh, hT[:, 128:256], whh1[:, 2 * H:3 * H],
                                     start=False, stop=True)

                r = wk.tile([BL, H], bf16, tag="r")
                nc.scalar.activation(r, grz[:, 0:H], AF.Sigmoid)
                z = wk.tile([BL, H], bf16, tag="z")
                nc.scalar.activation(z, grz[:, H:2 * H], AF.Sigmoid,
                                     bias=zbias[:, t:t + 1])
                zb = wk.tile([BL, H], bf16, tag="zb")
                nc.vector.scalar_tensor_tensor(zb, z, -1.0, onesb,
                                               op0=ALU.mult, op1=ALU.add)
                hTp = psT.tile([128, 2 * BL], f32, tag="hTp")
                if t > 0:
                    w1t = wk.tile([BL, H], f32, tag="w1t")
                    nc.gpsimd.tensor_mul(w1t, z, h_prev)
                    # transpose w1t into the hT accumulator early (off-chain)
                    nc.tensor.matmul(hTp[:, 0:BL], w1t[:, 0:128], eye32,
                                     is_transpose=True, start=True, stop=False)
                    nc.tensor.matmul(hTp[:, BL:2 * BL], w1t[:, 128:256], eye32,
                                     is_transpose=True, start=False, stop=False)

                n = wk.tile([BL, H], bf16, tag="n")
                if t > 0:
                    u = wk.tile([BL, H], bf16, tag="u")
                    nc.vector.tensor_mul(u, r, gnh)
                    nc.tensor.matmul(gni, eye16, u, start=False, stop=True)
                nc.scalar.activation(n, gni, AF.Tanh)

                w2t = wk.tile([BL, H], f32, tag="w2t")
                nc.vector.tensor_mul(w2t, zb, n)
                # h^T = T(w1t) + T(w2t) accumulated on PE
                nc.tensor.matmul(hTp[:, 0:BL], w2t[:, 0:128], eye32,
                                 is_transpose=True, start=(t == 0), stop=False)
                nc.tensor.matmul(hTp[:, BL:2 * BL], w2t[:, 128:256], eye32,
                                 is_transpose=True, start=False, stop=True)
                hT = wk.tile([128, 2 * BL], bf16, tag="hT")
                nc.vector.tensor_copy(hT[:, 0:BL], hTp[:, 0:BL])
                nc.vector.tensor_copy(hT[:, BL:2 * BL], hTp[:, BL:2 * BL])

                # h natural (off the recurrence chain): hid DMA + next-step w1t
                h = hwork.tile([BL, H], f32, tag="h")
                if t > 0:
                    nc.vector.tensor_add(h, w1t, w2t)
                else:
                    nc.vector.tensor_copy(h, w2t)
                h_prev = h
                hb = hring.tile([BL, H], bf16, tag="hb")
                nc.vector.tensor_copy(hb, h)
                h_hist.append(hb)

                # deferred attention/output block for the previous step
                if t > 0:
                    attn_out_block(t - 1, hT_prev_attn)
                hT_prev_attn = hT


            attn_out_block(L - 1, hT)

            # ---- deferred attention context ----
            Et = cp.tile([BL, L], f32, tag="Et")
            nc.scalar.activation(Et, scores, AF.Exp)
            ep = cp.tile([BL, L], f32, tag="ep")
            nc.vector.tensor_mul(ep, Et, maskt)
            d0 = cp.tile([BL, 1], f32, tag="d0")
            nc.vector.tensor_reduce(d0, ep, axis=mybir.AxisListType.X, op=ALU.add)
            dsum = cp.tile([BL, 1], f32, tag="dsum")
            nc.vector.tensor_add(dsum, d0, padd)
            rd = cp.tile([BL, 1], f32, tag="rd")
            nc.vector.reciprocal(rd, dsum)

            acc = psAcc.tile([BL, H], f32, tag="acc")
            for t in range(L):
                tmp = wk.tile([BL, H], bf16, tag="tmp")
                nc.vector.tensor_scalar_mul(tmp, h_hist[t], ep[:, t:t + 1])
                nc.tensor.matmul(acc, eye16, tmp, start=(t == 0), stop=(t == L - 1))

            ctx = cp.tile([BL, H], f32, tag="ctx")
            nc.vector.tensor_scalar_mul(ctx, acc, rd)

            # ---- cause MLP ----  (PSUM tiles reuse loop tags to stay in 8 banks)
            ctxTp = psS1.tile([128, 2 * BL], f32, tag="s1")
            nc.tensor.transpose(ctxTp[:, 0:BL], ctx[:, 0:128], eye32)
            nc.tensor.transpose(ctxTp[:, BL:2 * BL], ctx[:, 128:256], eye32)
            ctxT = cp.tile([128, 2 * BL], f32, tag="ctxT")
            nc.vector.tensor_copy(ctxT, ctxTp)

            zcp = psGrz.tile([BL, CH], f32, tag="grz")
            nc.tensor.matmul(zcp, ctxT[:, 0:BL], w1p[:, 0:CH], start=True, stop=False)
            nc.tensor.matmul(zcp, ctxT[:, BL:2 * BL], w1p[:, CH:2 * CH],
                             start=False, stop=False)
            nc.tensor.matmul(zcp, lastT, w1p[:, 2 * CH:3 * CH], start=False, stop=False)
            nc.tensor.matmul(zcp, ones1[:, 0:BL], b1r, start=False, stop=True)
            zc = cp.tile([BL, CH], f32, tag="zc")
            nc.scalar.activation(zc, zcp, AF.Relu)

            zcTp = psS1.tile([128, 2 * BL], f32, tag="s1")
            nc.tensor.transpose(zcTp[:, 0:BL], zc[:, 0:128], eye32)
            nc.tensor.transpose(zcTp[:, BL:2 * BL], zc[:, 128:256], eye32)
            zcT = cp.tile([128, 2 * BL], f32, tag="zcT")
            nc.vector.tensor_copy(zcT, zcTp)

            lg = psGrz.tile([BL, OUT], f32, tag="grz")
            nc.tensor.matmul(lg, zcT[:, 0:BL], w2p[:, 0:OUT], start=True, stop=False)
            nc.tensor.matmul(lg, zcT[:, BL:2 * BL], w2p[:, OUT:2 * OUT],
                             start=False, stop=False)
            nc.tensor.matmul(lg, ones1[:, 0:BL], b2r, start=False, stop=True)

            mx = cp.tile([BL, 1], f32, tag="mx")
            nc.vector.tensor_reduce(mx, lg, axis=mybir.AxisListType.X, op=ALU.max)
            nmx = cp.tile([BL, 1], f32, tag="nmx")
            nc.vector.tensor_scalar_mul(nmx, mx, -1.0)
            ex = cp.tile([BL, OUT], f32, tag="ex")
            sm = cp.tile([BL, 1], f32, tag="sm")
            nc.scalar.activation(ex, lg, AF.Exp, bias=nmx, accum_out=sm)
            rs = cp.tile([BL, 1], f32, tag="rs")
            nc.vector.reciprocal(rs, sm)
            fht = cp.tile([BL, OUT], f32, tag="fht")
            nc.vector.tensor_scalar_mul(fht, ex, rs)
            nc.sync.dma_start(out=d_fht[:], in_=fht)

    nc.finalize()
    _CACHE[key] = nc
    return nc


def _prep(input_batch, tte, W_ih, W_hh, b_ih, b_hh, W_out, b_out,
          Wa, Ua, va, W1, b1, W2, b2):
    input_batch = np.asarray(input_batch, np.float32)
    tte = np.asarray(tte, np.int32)
    W_ih = np.asarray(W_ih, np.float32)
    W_hh = np.asarray(W_hh, np.float32)
    b_ih = np.asarray(b_ih, np.float32)
    b_hh = np.asarray(b_hh, np.float32)
    W_out = np.asarray(W_out, np.float32)
    b_out = np.asarray(b_out, np.float32)
    Wa_ = np.asarray(Wa, np.float32)
    Ua = np.asarray(Ua, np.float32)
    va = np.asarray(va, np.float32)
    W1 = np.asarray(W1, np.float32)
    b1 = np.asarray(b1, np.float32)
    W2 = np.asarray(W2, np.float32)
    b2 = np.asarray(b2, np.float32)

    gb = b_ih + b_hh
    has_gbias = bool(np.any(gb))
    has_obias = bool(np.any(b_out))

    # shared (replicated) tensors
    wih_np = np.ascontiguousarray(W_ih.T).astype(BF16)              # [F, 3H]
    WhhT = W_hh.T                                                   # [H, 3H]
    whh_np = np.stack([WhhT[0:128], WhhT[128:256]]).astype(BF16)    # [2,128,3H]
    wa_np = np.concatenate([Wa_[0:128], Wa_[128:256]], axis=1).astype(BF16)
    WoutT = W_out.T                                                 # [H, F]
    woutT_np = np.concatenate([WoutT[0:128], WoutT[128:256]], axis=1).astype(BF16)
    w1_np = np.concatenate([W1[0:128], W1[128:256], W1[256:384]],
                           axis=1).astype(np.float32)               # [128, 3CH]
    w2_np = np.concatenate([W2[0:128], W2[128:256]], axis=1).astype(np.float32)
    b1_np = b1[None, :].astype(np.float32)
    b2_np = b2[None, :].astype(np.float32)
    eye16_np = np.eye(128, dtype=np.float32).astype(BF16)
    eye32_np = np.eye(128, dtype=np.float32)
    vab_np = np.broadcast_to(va, (BL, A)).astype(BF16).copy()
    gb_np = gb[None, :].astype(np.float32)
    ob_np = b_out[None, :].astype(np.float32)

    t_idx = np.arange(L, dtype=np.int32)[None, :]
    in_maps = []
    for c in range(NCORES):
        sl = slice(c * BL, (c + 1) * BL)
        xb = input_batch[sl]                      # [BL, L, F]
        ttec = tte[sl]
        last = xb[np.arange(BL), ttec]            # [BL, F]
        lastUa = last @ Ua                        # [BL, A]
        c_pad = np.tanh(lastUa) @ va              # [BL]
        pad_d = ((L - ttec).astype(np.float32) * np.exp(c_pad)).astype(np.float32)
        active = (t_idx < ttec[:, None]).astype(np.float32)  # [BL, L]
        m = {
            "xT": np.ascontiguousarray(xb.transpose(1, 2, 0)).astype(BF16),
            "wih": wih_np, "whh": whh_np, "wa": wa_np, "woutT": woutT_np,
            "lastua": lastUa.astype(BF16), "vab": vab_np,
            "mask": active, "zbias": (BIG * (1.0 - active)).astype(np.float32),
            "padd": pad_d[:, None],
            "lastT": np.ascontiguousarray(last.T).astype(np.float32),
            "w1p": w1_np, "w2p": w2_np, "b1r": b1_np, "b2r": b2_np,
            "eye16": eye16_np, "eye32": eye32_np,
        }
        if has_gbias:
            m["gbias"] = gb_np
        if has_obias:
            m["obias"] = ob_np
        in_maps.append(m)

    return in_maps, has_gbias, has_obias


def _prep_inputs(inputs):
    """Host-side preprocessing: returns (in_maps, has_gbias, has_obias)."""
    return _prep(**inputs)


def kernel(**inputs):
    in_maps, has_gbias, has_obias = _prep(**inputs)
    nc = _build(has_gbias, has_obias)
    res = run_bass_kernel_spmd(nc, in_maps, core_ids=list(range(NCORES)))
    outs = res.results
    output_batch = np.concatenate([r["out"] for r in outs], axis=0)
    fht = np.concatenate([r["fht"] for r in outs], axis=0)
    return output_batch.astype(np.float32), fht.astype(np.float32)


# revision 42
# speedup vs baseline: 1.1895x; 1.0154x over previous
"""DynamicDeepHit (GRU + additive attention + cause MLP) Bass kernel for 8 trn2 cores.

Sharding: pure data-parallel over batch B=1024 -> 128 per core; weights replicated.

Per-core device program (SPMD, identical on all 8 cores), batch-major layout
[B=128 partitions, features]; L=256 steps fully unrolled:
  - x_t^T loaded pre-transposed from DRAM (host passes [L, F, B] bf16)
  - G = x@W_ihT + h@W_hhT on PE (lhsT = x^T / h^T stationary, f32 PSUM accum);
    x-parts are emitted first so PE runs them while waiting on the recurrence
  - r = sigmoid(Gr); z = sigmoid(Gz + 30*(1-active_t)) — the ragged-sequence
    freeze (h stops updating at t >= tte) is folded into the z gate bias
  - n = tanh(Gni + r*Gnh): r*Gnh on DVE reading PSUM, added into the Gni bank
    via an identity-matmul so tanh reads one PSUM tile
  - h update split as w1 = z*h_old (GPSIMD), w2 = (1-z)*n (DVE); h^T is formed
    by PE transpose-accumulate of w1 and w2 directly in PSUM (keeps the h-add
    off the recurrence critical path); both PSUM->SBUF copies on DVE
  - h written bf16 straight into a 256-deep SBUF ring (attention phase reads it)
  - attention score s_t = tanh(h@Wa + last@Ua) . va (PE + ACT tanh + fused
    scalar_tensor_tensor mul-reduce); scores from frozen h are fixed up
    post-loop, so no per-step masking is needed
  - out_t = active_t * (h@W_outT) -> DMA (masking via per-partition scalar)
Post-loop: E = exp(scores); e' = E*mask; d = sum(e') + pad_d where pad_d is the
host-computed softmax mass of the padded positions ((L-tte)*exp(tanh(lastUa).va));
context = (sum_t e'_t * h_t)/d accumulated via PE identity-matmuls; then the
cause MLP + softmax -> fht.

Host side precomputes: transposed/packed weights (bf16 for matmuls), last
(gather at tte), lastUa, active mask, z-freeze bias, pad_d, identities.

PSUM budget (8 banks, one accumulation group per bank at a time):
  grz x2 (double-buffered), gni, gnh, s1, o, hTp, acc.
"""

import numpy as np
import sys

for _p in ("/opt/trn_rl_repo", "/root/.axon_site/_ro/trn_rl_repo"):
    if _p not in sys.path:
        sys.path.insert(0, _p)

import ml_dtypes

import concourse.bacc as bacc
import concourse.bass as bass
import concourse.tile as tile
from concourse import mybir
from concourse.bass_utils import run_bass_kernel_spmd

BF16 = ml_dtypes.bfloat16

# Problem dims (hardcoded per spec)
B, L, F, H = 1024, 256, 128, 256
A, CH, OUT = 128, 256, 512
NCORES = 8
BL = B // NCORES  # 128 per core
BIG = 30.0

f32 = mybir.dt.float32
bf16 = mybir.dt.bfloat16
AF = mybir.ActivationFunctionType
ALU = mybir.AluOpType

_CACHE = {}


def _build(has_gbias: bool, has_obias: bool):
    key = (has_gbias, has_obias)
    if key in _CACHE:
        return _CACHE[key]

    nc = bacc.Bacc("TRN2", target_bir_lowering=False, debug=False,
                   num_devices=NCORES)

    # ---- DRAM I/O (per-core shapes) ----
    d_xT = nc.dram_tensor("xT", [L, F, BL], bf16, kind="ExternalInput")
    d_wih = nc.dram_tensor("wih", [F, 3 * H], bf16, kind="ExternalInput")
    d_whh = nc.dram_tensor("whh", [2, 128, 3 * H], bf16, kind="ExternalInput")
    d_wa = nc.dram_tensor("wa", [128, 2 * A], bf16, kind="ExternalInput")
    d_woutT = nc.dram_tensor("woutT", [128, 2 * F], bf16, kind="ExternalInput")
    d_lastua = nc.dram_tensor("lastua", [BL, A], bf16, kind="ExternalInput")
    d_vab = nc.dram_tensor("vab", [BL, A], bf16, kind="ExternalInput")
    d_mask = nc.dram_tensor("mask", [BL, L], f32, kind="ExternalInput")
    d_zbias = nc.dram_tensor("zbias", [BL, L], f32, kind="ExternalInput")
    d_padd = nc.dram_tensor("padd", [BL, 1], f32, kind="ExternalInput")
    d_lastT = nc.dram_tensor("lastT", [F, BL], f32, kind="ExternalInput")
    d_w1 = nc.dram_tensor("w1p", [128, 3 * CH], f32, kind="ExternalInput")
    d_w2 = nc.dram_tensor("w2p", [128, 2 * OUT], f32, kind="ExternalInput")
    d_b1 = nc.dram_tensor("b1r", [1, CH], f32, kind="ExternalInput")
    d_b2 = nc.dram_tensor("b2r", [1, OUT], f32, kind="ExternalInput")
    d_eye16 = nc.dram_tensor("eye16", [128, 128], bf16, kind="ExternalInput")
    d_eye32 = nc.dram_tensor("eye32", [128, 128], f32, kind="ExternalInput")
    if has_gbias:
        d_gbias = nc.dram_tensor("gbias", [1, 3 * H], f32, kind="ExternalInput")
    if has_obias:
        d_obias = nc.dram_tensor("obias", [1, F], f32, kind="ExternalInput")

    d_out = nc.dram_tensor("out", [BL, L, F], f32, kind="ExternalOutput")
    d_fht = nc.dram_tensor("fht", [BL, OUT], f32, kind="ExternalOutput")

    with tile.TileContext(nc) as tc:
        with (
            tc.tile_pool(name="const", bufs=1) as cp,
            tc.tile_pool(name="hwork", bufs=4) as hwork,
            tc.tile_pool(name="hring", bufs=L + 2) as hring,
            tc.tile_pool(name="work", bufs=4) as wk,
            tc.tile_pool(name="xin", bufs=6) as xin,
            tc.tile_pool(name="ostage", bufs=6) as ost,
            tc.tile_pool(name="psGrz", bufs=2, space="PSUM") as psGrz,
            tc.tile_pool(name="psGni", bufs=1, space="PSUM") as psGni,
            tc.tile_pool(name="psGnh", bufs=1, space="PSUM") as psGnh,
            tc.tile_pool(name="psS1", bufs=1, space="PSUM") as psS1,
            tc.tile_pool(name="psO", bufs=1, space="PSUM") as psO,
            tc.tile_pool(name="psT", bufs=1, space="PSUM") as psT,
            tc.tile_pool(name="psAcc", bufs=1, space="PSUM") as psAcc,
        ):
            # ---- constants into SBUF ----
            wih = cp.tile([F, 3 * H], bf16, tag="wih")
            nc.sync.dma_start(out=wih, in_=d_wih[:])
            whh0 = cp.tile([128, 3 * H], bf16, tag="whh0")
            nc.sync.dma_start(out=whh0, in_=d_whh[0])
            whh1 = cp.tile([128, 3 * H], bf16, tag="whh1")
            nc.sync.dma_start(out=whh1, in_=d_whh[1])
            wa = cp.tile([128, 2 * A], bf16, tag="wa")
            nc.sync.dma_start(out=wa, in_=d_wa[:])
            woutT = cp.tile([128, 2 * F], bf16, tag="woutT")
            nc.sync.dma_start(out=woutT, in_=d_woutT[:])
            lastua = cp.tile([BL, A], bf16, tag="lastua")
            nc.sync.dma_start(out=lastua, in_=d_lastua[:])
            vab = cp.tile([BL, A], bf16, tag="vab")
            nc.sync.dma_start(out=vab, in_=d_vab[:])
            maskt = cp.tile([BL, L], f32, tag="mask")
            nc.sync.dma_start(out=maskt, in_=d_mask[:])
            zbias = cp.tile([BL, L], f32, tag="zbias")
            nc.sync.dma_start(out=zbias, in_=d_zbias[:])
            padd = cp.tile([BL, 1], f32, tag="padd")
            nc.sync.dma_start(out=padd, in_=d_padd[:])
            lastT = cp.tile([F, BL], f32, tag="lastT")
            nc.sync.dma_start(out=lastT, in_=d_lastT[:])
            w1p = cp.tile([128, 3 * CH], f32, tag="w1p")
            nc.sync.dma_start(out=w1p, in_=d_w1[:])
            w2p = cp.tile([128, 2 * OUT], f32, tag="w2p")
            nc.sync.dma_start(out=w2p, in_=d_w2[:])
            b1r = cp.tile([1, CH], f32, tag="b1r")
            nc.sync.dma_start(out=b1r, in_=d_b1[:])
            b2r = cp.tile([1, OUT], f32, tag="b2r")
            nc.sync.dma_start(out=b2r, in_=d_b2[:])
            eye16 = cp.tile([128, 128], bf16, tag="eye16")
            nc.sync.dma_start(out=eye16, in_=d_eye16[:])
            eye32 = cp.tile([128, 128], f32, tag="eye32")
            nc.sync.dma_start(out=eye32, in_=d_eye32[:])
            ones1 = cp.tile([1, 128], f32, tag="ones1")
            nc.vector.memset(ones1, 1.0)
            onesb = cp.tile([BL, H], bf16, tag="onesb")
            nc.vector.memset(onesb, 1.0)
            if has_gbias:
                gbias = cp.tile([1, 3 * H], f32, tag="gbias")
                nc.sync.dma_start(out=gbias, in_=d_gbias[:])
            if has_obias:
                obias = cp.tile([1, F], f32, tag="obias")
                nc.sync.dma_start(out=obias, in_=d_obias[:])

            scores = cp.tile([BL, L], f32, tag="scores")

            # ---- GRU loop ----
            # Emission order per iteration: G-matmuls for step t first (so the
            # PE queue prioritizes the recurrence), then the deferred
            # attention/output block for step t-1, then gate math for t.
            hT = None
            h_prev = None
            h_hist = []

            def attn_out_block(t, hT_t):
                # S1 = lastUa + h@Wa ; s_t = tanh(S1).va ; out_t = mask*(h@WoutT)
                s1 = psS1.tile([BL, A], f32, tag="s1", name=f"s1_{t}")
                nc.tensor.matmul(s1, eye16, lastua, start=True, stop=False)
                nc.tensor.matmul(s1, hT_t[:, 0:128], wa[:, 0:A],
                                 start=False, stop=False)
                nc.tensor.matmul(s1, hT_t[:, 128:256], wa[:, A:2 * A],
                                 start=False, stop=True)
                o = psO.tile([BL, F], f32, tag="o", name=f"o_{t}")
                nc.tensor.matmul(o, hT_t[:, 0:128], woutT[:, 0:F],
                                 start=True, stop=False)
                nc.tensor.matmul(o, hT_t[:, 128:256], woutT[:, F:2 * F],
                                 start=False, stop=not has_obias)
                if has_obias:
                    nc.tensor.matmul(o, ones1[:, 0:BL], obias,
                                     start=False, stop=True)
                tanha = wk.tile([BL, A], bf16, tag="tanha", name=f"tanha_{t}")
                nc.scalar.activation(tanha, s1, AF.Tanh)
                scratch = wk.tile([BL, A], bf16, tag="scratch", name=f"scr_{t}")
                nc.vector.scalar_tensor_tensor(
                    scratch, tanha, 1.0, vab, op0=ALU.mult, op1=ALU.mult,
                    accum_out=scores[:, t:t + 1])
                ot = ost.tile([BL, F], f32, tag="ot", name=f"ot_{t}")
                nc.vector.tensor_scalar_mul(ot, o, maskt[:, t:t + 1])
                nc.sync.dma_start(out=d_out[:, t, :], in_=ot)

            for t in range(L):
                xt = xin.tile([F, BL], bf16, tag="xt")
                nc.sync.dma_start(out=xt, in_=d_xT[t])

                grz = psGrz.tile([BL, 2 * H], f32, tag="grz")
                gni = psGni.tile([BL, H], f32, tag="gni")
                if t > 0:
                    gnh = psGnh.tile([BL, H], f32, tag="gnh")
                else:
                    gnh = None

                # x contributions first (no hT dependency; PE can run them early)
                nc.tensor.matmul(grz, xt, wih[:, 0:2 * H], start=True,
                                 stop=(t == 0) and not has_gbias)
                gni_instant = (t == 0) and not has_gbias
                nc.tensor.matmul(gni, xt, wih[:, 2 * H:3 * H],
                                 start=True, stop=gni_instant)
                if has_gbias:
                    nc.tensor.matmul(grz, ones1[:, 0:BL], gbias[:, 0:2 * H],
                                     start=False, stop=(t == 0))
                    nc.tensor.matmul(gni, ones1[:, 0:BL], gbias[:, 2 * H:3 * H],
                                     start=False, stop=(t == 0))
                # h contributions
                if t > 0:
                    nc.tensor.matmul(grz, hT[:, 0:128], whh0[:, 0:2 * H],
                                     start=False, stop=False)
                    nc.tensor.matmul(grz, hT[:, 128:256], whh1[:, 0:2 * H],
                                     start=False, stop=True)
                    nc.tensor.matmul(gnh, hT[:, 0:128], whh0[:, 2 * H:3 * H],
                                     start=True, stop=False)
                    nc.tensor.matmul(gnh, hT[:, 128:256], whh1[:, 2 * H:3 * H],
                                     start=False, stop=True)

                r = wk.tile([BL, H], bf16, tag="r")
                nc.scalar.activation(r, grz[:, 0:H], AF.Sigmoid)
                z = wk.tile([BL, H], bf16, tag="z")
                nc.scalar.activation(z, grz[:, H:2 * H], AF.Sigmoid,
                                     bias=zbias[:, t:t + 1])
                zb = wk.tile([BL, H], bf16, tag="zb")
                nc.vector.scalar_tensor_tensor(zb, z, -1.0, onesb,
                                               op0=ALU.mult, op1=ALU.add)
                hTp = psT.tile([128, 2 * BL], f32, tag="hTp")
                if t > 0:
                    w1t = wk.tile([BL, H], f32, tag="w1t")
                    nc.gpsimd.tensor_mul(w1t, z, h_prev)
                    # transpose w1t into the hT accumulator early (off-chain)
                    nc.tensor.matmul(hTp[:, 0:BL], w1t[:, 0:128], eye32,
                                     is_transpose=True, start=True, stop=False)
                    nc.tensor.matmul(hTp[:, BL:2 * BL], w1t[:, 128:256], eye32,
                                     is_transpose=True, start=False, stop=False)

                n = wk.tile([BL, H], bf16, tag="n")
                if t > 0:
                    u = wk.tile([BL, H], bf16, tag="u")
                    nc.vector.tensor_mul(u, r, gnh)
                    nc.tensor.matmul(gni, eye16, u, start=False, stop=True)
                nc.scalar.activation(n, gni, AF.Tanh)

                w2t = wk.tile([BL, H], f32, tag="w2t")
                nc.vector.tensor_mul(w2t, zb, n)
                # h^T = T(w1t) + T(w2t) accumulated on PE
                nc.tensor.matmul(hTp[:, 0:BL], w2t[:, 0:128], eye32,
                                 is_transpose=True, start=(t == 0), stop=False)
                nc.tensor.matmul(hTp[:, BL:2 * BL], w2t[:, 128:256], eye32,
                                 is_transpose=True, start=False, stop=True)
                hT = wk.tile([128, 2 * BL], bf16, tag="hT")
                nc.vector.tensor_copy(hT[:, 0:BL], hTp[:, 0:BL])
                nc.vector.tensor_copy(hT[:, BL:2 * BL], hTp[:, BL:2 * BL])

                # h natural (off the recurrence chain), stored bf16 in the ring
                h = hring.tile([BL, H], bf16, tag="hb")
                if t > 0:
                    nc.vector.tensor_add(h, w1t, w2t)
                else:
                    nc.vector.tensor_copy(h, w2t)
                h_prev = h
                h_hist.append(h)

                # deferred attention/output block for the previous step
                if t > 0:
                    attn_out_block(t - 1, hT_prev_attn)
                hT_prev_attn = hT


            attn_out_block(L - 1, hT)

            # ---- deferred attention context ----
            Et = cp.tile([BL, L], f32, tag="Et")
            nc.scalar.activation(Et, scores, AF.Exp)
            ep = cp.tile([BL, L], f32, tag="ep")
            nc.vector.tensor_mul(ep, Et, maskt)
            d0 = cp.tile([BL, 1], f32, tag="d0")
            nc.vector.tensor_reduce(d0, ep, axis=mybir.AxisListType.X, op=ALU.add)
            dsum = cp.tile([BL, 1], f32, tag="dsum")
            nc.vector.tensor_add(dsum, d0, padd)
            rd = cp.tile([BL, 1], f32, tag="rd")
            nc.vector.reciprocal(rd, dsum)

            acc = psAcc.tile([BL, H], f32, tag="acc")
            for t in range(L):
                tmp = wk.tile([BL, H], bf16, tag="tmp")
                nc.vector.tensor_scalar_mul(tmp, h_hist[t], ep[:, t:t + 1])
                nc.tensor.matmul(acc, eye16, tmp, start=(t == 0), stop=(t == L - 1))

            ctx = cp.tile([BL, H], f32, tag="ctx")
            nc.vector.tensor_scalar_mul(ctx, acc, rd)

            # ---- cause MLP ----  (PSUM tiles reuse loop tags to stay in 8 banks)
            ctxTp = psS1.tile([128, 2 * BL], f32, tag="s1")
            nc.tensor.transpose(ctxTp[:, 0:BL], ctx[:, 0:128], eye32)
            nc.tensor.transpose(ctxTp[:, BL:2 * BL], ctx[:, 128:256], eye32)
            ctxT = cp.tile([128, 2 * BL], f32, tag="ctxT")
            nc.vector.tensor_copy(ctxT, ctxTp)

            zcp = psGrz.tile([BL, CH], f32, tag="grz")
            nc.tensor.matmul(zcp, ctxT[:, 0:BL], w1p[:, 0:CH], start=True, stop=False)
            nc.tensor.matmul(zcp, ctxT[:, BL:2 * BL], w1p[:, CH:2 * CH],
                             start=False, stop=False)
            nc.tensor.matmul(zcp, lastT, w1p[:, 2 * CH:3 * CH], start=False, stop=False)
            nc.tensor.matmul(zcp, ones1[:, 0:BL], b1r, start=False, stop=True)
            zc = cp.tile([BL, CH], f32, tag="zc")
            nc.scalar.activation(zc, zcp, AF.Relu)

            zcTp = psS1.tile([128, 2 * BL], f32, tag="s1")
            nc.tensor.transpose(zcTp[:, 0:BL], zc[:, 0:128], eye32)
            nc.tensor.transpose(zcTp[:, BL:2 * BL], zc[:, 128:256], eye32)
            zcT = cp.tile([128, 2 * BL], f32, tag="zcT")
            nc.vector.tensor_copy(zcT, zcTp)

            lg = psGrz.tile([BL, OUT], f32, tag="grz")
            nc.tensor.matmul(lg, zcT[:, 0:BL], w2p[:, 0:OUT], start=True, stop=False)
            nc.tensor.matmul(lg, zcT[:, BL:2 * BL], w2p[:, OUT:2 * OUT],
                             start=False, stop=False)
            nc.tensor.matmul(lg, ones1[:, 0:BL], b2r, start=False, stop=True)

            mx = cp.tile([BL, 1], f32, tag="mx")
            nc.vector.tensor_reduce(mx, lg, axis=mybir.AxisListType.X, op=ALU.max)
            nmx = cp.tile([BL, 1], f32, tag="nmx")
            nc.vector.tensor_scalar_mul(nmx, mx, -1.0)
            ex = cp.tile([BL, OUT], f32, tag="ex")
            sm = cp.tile([BL, 1], f32, tag="sm")
            nc.scalar.activation(ex, lg, AF.Exp, bias=nmx, accum_out=sm)
            rs = cp.tile([BL, 1], f32, tag="rs")
            nc.vector.reciprocal(rs, sm)
            fht = cp.tile([BL, OUT], f32, tag="fht")
            nc.vector.tensor_scalar_mul(fht, ex, rs)
            nc.sync.dma_start(out=d_fht[:], in_=fht)

    nc.finalize()
    _CACHE[key] = nc
    return nc


def _prep(input_batch, tte, W_ih, W_hh, b_ih, b_hh, W_out, b_out,
          Wa, Ua, va, W1, b1, W2, b2):
    input_batch = np.asarray(input_batch, np.float32)
    tte = np.asarray(tte, np.int32)
    W_ih = np.asarray(W_ih, np.float32)
    W_hh = np.asarray(W_hh, np.float32)
    b_ih = np.asarray(b_ih, np.float32)
    b_hh = np.asarray(b_hh, np.float32)
    W_out = np.asarray(W_out, np.float32)
    b_out = np.asarray(b_out, np.float32)
    Wa_ = np.asarray(Wa, np.float32)
    Ua = np.asarray(Ua, np.float32)
    va = np.asarray(va, np.float32)
    W1 = np.asarray(W1, np.float32)
    b1 = np.asarray(b1, np.float32)
    W2 = np.asarray(W2, np.float32)
    b2 = np.asarray(b2, np.float32)

    gb = b_ih + b_hh
    has_gbias = bool(np.any(gb))
    has_obias = bool(np.any(b_out))

    # shared (replicated) tensors
    wih_np = np.ascontiguousarray(W_ih.T).astype(BF16)              # [F, 3H]
    WhhT = W_hh.T                                                   # [H, 3H]
    whh_np = np.stack([WhhT[0:128], WhhT[128:256]]).astype(BF16)    # [2,128,3H]
    wa_np = np.concatenate([Wa_[0:128], Wa_[128:256]], axis=1).astype(BF16)
    WoutT = W_out.T                                                 # [H, F]
    woutT_np = np.concatenate([WoutT[0:128], WoutT[128:256]], axis=1).astype(BF16)
    w1_np = np.concatenate([W1[0:128], W1[128:256], W1[256:384]],
                           axis=1).astype(np.float32)               # [128, 3CH]
    w2_np = np.concatenate([W2[0:128], W2[128:256]], axis=1).astype(np.float32)
    b1_np = b1[None, :].astype(np.float32)
    b2_np = b2[None, :].astype(np.float32)
    eye16_np = np.eye(128, dtype=np.float32).astype(BF16)
    eye32_np = np.eye(128, dtype=np.float32)
    vab_np = np.broadcast_to(va, (BL, A)).astype(BF16).copy()
    gb_np = gb[None, :].astype(np.float32)
    ob_np = b_out[None, :].astype(np.float32)

    t_idx = np.arange(L, dtype=np.int32)[None, :]
    in_maps = []
    for c in range(NCORES):
        sl = slice(c * BL, (c + 1) * BL)
        xb = input_batch[sl]                      # [BL, L, F]
        ttec = tte[sl]
        last = xb[np.arange(BL), ttec]            # [BL, F]
        lastUa = last @ Ua                        # [BL, A]
        c_pad = np.tanh(lastUa) @ va              # [BL]
        pad_d = ((L - ttec).astype(np.float32) * np.exp(c_pad)).astype(np.float32)
        active = (t_idx < ttec[:, None]).astype(np.float32)  # [BL, L]
        m = {
            "xT": np.ascontiguousarray(xb.transpose(1, 2, 0)).astype(BF16),
            "wih": wih_np, "whh": whh_np, "wa": wa_np, "woutT": woutT_np,
            "lastua": lastUa.astype(BF16), "vab": vab_np,
            "mask": active, "zbias": (BIG * (1.0 - active)).astype(np.float32),
            "padd": pad_d[:, None],
            "lastT": np.ascontiguousarray(last.T).astype(np.float32),
            "w1p": w1_np, "w2p": w2_np, "b1r": b1_np, "b2r": b2_np,
            "eye16": eye16_np, "eye32": eye32_np,
        }
        if has_gbias:
            m["gbias"] = gb_np
        if has_obias:
            m["obias"] = ob_np
        in_maps.append(m)

    return in_maps, has_gbias, has_obias


def _prep_inputs(inputs):
    """Host-side preprocessing: returns (in_maps, has_gbias, has_obias)."""
    return _prep(**inputs)


def kernel(**inputs):
    in_maps, has_gbias, has_obias = _prep(**inputs)
    nc = _build(has_gbias, has_obias)
    res = run_bass_kernel_spmd(nc, in_maps, core_ids=list(range(NCORES)))
    outs = res.results
    output_batch = np.concatenate([r["out"] for r in outs], axis=0)
    fht = np.concatenate([r["fht"] for r in outs], axis=0)
    return output_batch.astype(np.float32), fht.astype(np.float32)


# revision 43
# speedup vs baseline: 1.1964x; 1.0057x over previous
"""DynamicDeepHit (GRU + additive attention + cause MLP) Bass kernel for 8 trn2 cores.

Sharding: pure data-parallel over batch B=1024 -> 128 per core; weights replicated.

Per-core device program (SPMD, identical on all 8 cores), batch-major layout
[B=128 partitions, features]; L=256 steps fully unrolled:
  - x_t^T loaded pre-transposed from DRAM (host passes [L, F, B] bf16)
  - G = x@W_ihT + h@W_hhT on PE (lhsT = x^T / h^T stationary, f32 PSUM accum);
    x-parts are emitted first so PE runs them while waiting on the recurrence
  - r = sigmoid(Gr); z = sigmoid(Gz + 30*(1-active_t)) — the ragged-sequence
    freeze (h stops updating at t >= tte) is folded into the z gate bias
  - n = tanh(Gni + r*Gnh): r*Gnh on DVE reading PSUM, added into the Gni bank
    via an identity-matmul so tanh reads one PSUM tile
  - h update split as w1 = z*h_old (GPSIMD), w2 = (1-z)*n (DVE); h^T is formed
    by PE transpose-accumulate of w1 and w2 directly in PSUM (keeps the h-add
    off the recurrence critical path); both PSUM->SBUF copies on DVE
  - h written bf16 straight into a 256-deep SBUF ring (attention phase reads it)
  - attention score s_t = tanh(h@Wa + last@Ua) . va (PE + ACT tanh + fused
    scalar_tensor_tensor mul-reduce); scores from frozen h are fixed up
    post-loop, so no per-step masking is needed
  - out_t = active_t * (h@W_outT) -> DMA (masking via per-partition scalar)
Post-loop: E = exp(scores); e' = E*mask; d = sum(e') + pad_d where pad_d is the
host-computed softmax mass of the padded positions ((L-tte)*exp(tanh(lastUa).va));
context = (sum_t e'_t * h_t)/d accumulated via PE identity-matmuls; then the
cause MLP + softmax -> fht.

Host side precomputes: transposed/packed weights (bf16 for matmuls), last
(gather at tte), lastUa, active mask, z-freeze bias, pad_d, identities.

PSUM budget (8 banks, one accumulation group per bank at a time):
  grz x2 (double-buffered), gni, gnh, s1, o, hTp, acc.
"""

import numpy as np
import sys

for _p in ("/opt/trn_rl_repo", "/root/.axon_site/_ro/trn_rl_repo"):
    if _p not in sys.path:
        sys.path.insert(0, _p)

import ml_dtypes

import concourse.bacc as bacc
import concourse.bass as bass
import concourse.tile as tile
from concourse import mybir
from concourse.bass_utils import run_bass_kernel_spmd

BF16 = ml_dtypes.bfloat16

# Problem dims (hardcoded per spec)
B, L, F, H = 1024, 256, 128, 256
A, CH, OUT = 128, 256, 512
NCORES = 8
BL = B // NCORES  # 128 per core
BIG = 30.0

f32 = mybir.dt.float32
bf16 = mybir.dt.bfloat16
AF = mybir.ActivationFunctionType
ALU = mybir.AluOpType

_CACHE = {}


def _build(has_gbias: bool, has_obias: bool):
    key = (has_gbias, has_obias)
    if key in _CACHE:
        return _CACHE[key]

    nc = bacc.Bacc("TRN2", target_bir_lowering=False, debug=False,
                   num_devices=NCORES)

    # ---- DRAM I/O (per-core shapes) ----
    d_xT = nc.dram_tensor("xT", [L, F, BL], bf16, kind="ExternalInput")
    d_wih = nc.dram_tensor("wih", [F, 3 * H], bf16, kind="ExternalInput")
    d_whh = nc.dram_tensor("whh", [2, 128, 3 * H], bf16, kind="ExternalInput")
    d_wa = nc.dram_tensor("wa", [128, 2 * A], bf16, kind="ExternalInput")
    d_woutT = nc.dram_tensor("woutT", [128, 2 * F], bf16, kind="ExternalInput")
    d_lastua = nc.dram_tensor("lastua", [BL, A], bf16, kind="ExternalInput")
    d_vab = nc.dram_tensor("vab", [BL, A], bf16, kind="ExternalInput")
    d_mask = nc.dram_tensor("mask", [BL, L], f32, kind="ExternalInput")
    d_zbias = nc.dram_tensor("zbias", [BL, L], f32, kind="ExternalInput")
    d_padd = nc.dram_tensor("padd", [BL, 1], f32, kind="ExternalInput")
    d_lastT = nc.dram_tensor("lastT", [F, BL], f32, kind="ExternalInput")
    d_w1 = nc.dram_tensor("w1p", [128, 3 * CH], f32, kind="ExternalInput")
    d_w2 = nc.dram_tensor("w2p", [128, 2 * OUT], f32, kind="ExternalInput")
    d_b1 = nc.dram_tensor("b1r", [1, CH], f32, kind="ExternalInput")
    d_b2 = nc.dram_tensor("b2r", [1, OUT], f32, kind="ExternalInput")
    d_eye16 = nc.dram_tensor("eye16", [128, 128], bf16, kind="ExternalInput")
    d_eye32 = nc.dram_tensor("eye32", [128, 128], f32, kind="ExternalInput")
    if has_gbias:
        d_gbias = nc.dram_tensor("gbias", [1, 3 * H], f32, kind="ExternalInput")
    if has_obias:
        d_obias = nc.dram_tensor("obias", [1, F], f32, kind="ExternalInput")

    d_out = nc.dram_tensor("out", [BL, L, F], f32, kind="ExternalOutput")
    d_fht = nc.dram_tensor("fht", [BL, OUT], f32, kind="ExternalOutput")

    with tile.TileContext(nc) as tc:
        with (
            tc.tile_pool(name="const", bufs=1) as cp,
            tc.tile_pool(name="hwork", bufs=4) as hwork,
            tc.tile_pool(name="hring", bufs=L + 2) as hring,
            tc.tile_pool(name="work", bufs=4) as wk,
            tc.tile_pool(name="xin", bufs=6) as xin,
            tc.tile_pool(name="ostage", bufs=6) as ost,
            tc.tile_pool(name="psGrz", bufs=2, space="PSUM") as psGrz,
            tc.tile_pool(name="psGni", bufs=1, space="PSUM") as psGni,
            tc.tile_pool(name="psGnh", bufs=1, space="PSUM") as psGnh,
            tc.tile_pool(name="psS1", bufs=1, space="PSUM") as psS1,
            tc.tile_pool(name="psO", bufs=1, space="PSUM") as psO,
            tc.tile_pool(name="psT", bufs=1, space="PSUM") as psT,
            tc.tile_pool(name="psAcc", bufs=1, space="PSUM") as psAcc,
        ):
            # ---- constants into SBUF ----
            wih = cp.tile([F, 3 * H], bf16, tag="wih")
            nc.sync.dma_start(out=wih, in_=d_wih[:])
            whh0 = cp.tile([128, 3 * H], bf16, tag="whh0")
            nc.sync.dma_start(out=whh0, in_=d_whh[0])
            whh1 = cp.tile([128, 3 * H], bf16, tag="whh1")
            nc.sync.dma_start(out=whh1, in_=d_whh[1])
            wa = cp.tile([128, 2 * A], bf16, tag="wa")
            nc.sync.dma_start(out=wa, in_=d_wa[:])
            woutT = cp.tile([128, 2 * F], bf16, tag="woutT")
            nc.sync.dma_start(out=woutT, in_=d_woutT[:])
            lastua = cp.tile([BL, A], bf16, tag="lastua")
            nc.sync.dma_start(out=lastua, in_=d_lastua[:])
            vab = cp.tile([BL, A], bf16, tag="vab")
            nc.sync.dma_start(out=vab, in_=d_vab[:])
            maskt = cp.tile([BL, L], f32, tag="mask")
            nc.sync.dma_start(out=maskt, in_=d_mask[:])
            zbias = cp.tile([BL, L], f32, tag="zbias")
            nc.sync.dma_start(out=zbias, in_=d_zbias[:])
            padd = cp.tile([BL, 1], f32, tag="padd")
            nc.sync.dma_start(out=padd, in_=d_padd[:])
            lastT = cp.tile([F, BL], f32, tag="lastT")
            nc.sync.dma_start(out=lastT, in_=d_lastT[:])
            w1p = cp.tile([128, 3 * CH], f32, tag="w1p")
            nc.sync.dma_start(out=w1p, in_=d_w1[:])
            w2p = cp.tile([128, 2 * OUT], f32, tag="w2p")
            nc.sync.dma_start(out=w2p, in_=d_w2[:])
            b1r = cp.tile([1, CH], f32, tag="b1r")
            nc.sync.dma_start(out=b1r, in_=d_b1[:])
            b2r = cp.tile([1, OUT], f32, tag="b2r")
            nc.sync.dma_start(out=b2r, in_=d_b2[:])
            eye16 = cp.tile([128, 128], bf16, tag="eye16")
            nc.sync.dma_start(out=eye16, in_=d_eye16[:])
            eye32 = cp.tile([128, 128], f32, tag="eye32")
            nc.sync.dma_start(out=eye32, in_=d_eye32[:])
            ones1 = cp.tile([1, 128], f32, tag="ones1")
            nc.vector.memset(ones1, 1.0)
            onesb = cp.tile([BL, H], bf16, tag="onesb")
            nc.vector.memset(onesb, 1.0)
            if has_gbias:
                gbias = cp.tile([1, 3 * H], f32, tag="gbias")
                nc.sync.dma_start(out=gbias, in_=d_gbias[:])
            if has_obias:
                obias = cp.tile([1, F], f32, tag="obias")
                nc.sync.dma_start(out=obias, in_=d_obias[:])

            scores = cp.tile([BL, L], f32, tag="scores")

            # ---- GRU loop ----
            # Emission order per iteration: G-matmuls for step t first (so the
            # PE queue prioritizes the recurrence), then the deferred
            # attention/output block for step t-1, then gate math for t.
            hT = None
            h_prev = None
            h_hist = []

            def attn_out_block(t, hT_t):
                # S1 = lastUa + h@Wa ; s_t = tanh(S1).va ; out_t = mask*(h@WoutT)
                s1 = psS1.tile([BL, A], f32, tag="s1", name=f"s1_{t}")
                nc.tensor.matmul(s1, eye16, lastua, start=True, stop=False)
                nc.tensor.matmul(s1, hT_t[:, 0:128], wa[:, 0:A],
                                 start=False, stop=False)
                nc.tensor.matmul(s1, hT_t[:, 128:256], wa[:, A:2 * A],
                                 start=False, stop=True)
                o = psO.tile([BL, F], f32, tag="o", name=f"o_{t}")
                nc.tensor.matmul(o, hT_t[:, 0:128], woutT[:, 0:F],
                                 start=True, stop=False)
                nc.tensor.matmul(o, hT_t[:, 128:256], woutT[:, F:2 * F],
                                 start=False, stop=not has_obias)
                if has_obias:
                    nc.tensor.matmul(o, ones1[:, 0:BL], obias,
                                     start=False, stop=True)
                tanha = wk.tile([BL, A], bf16, tag="tanha", name=f"tanha_{t}")
                nc.scalar.activation(tanha, s1, AF.Tanh)
                scratch = wk.tile([BL, A], bf16, tag="scratch", name=f"scr_{t}")
                nc.vector.scalar_tensor_tensor(
                    scratch, tanha, 1.0, vab, op0=ALU.mult, op1=ALU.mult,
                    accum_out=scores[:, t:t + 1])
                ot = ost.tile([BL, F], f32, tag="ot", name=f"ot_{t}")
                nc.vector.tensor_scalar_mul(ot, o, maskt[:, t:t + 1])
                nc.sync.dma_start(out=d_out[:, t, :], in_=ot)

            for t in range(L):
                xt = xin.tile([F, BL], bf16, tag="xt")
                nc.sync.dma_start(out=xt, in_=d_xT[t])

                grz = psGrz.tile([BL, 2 * H], f32, tag="grz")
                gni = psGni.tile([BL, H], f32, tag="gni")
                if t > 0:
                    gnh = psGnh.tile([BL, H], f32, tag="gnh")
                else:
                    gnh = None

                # x contributions first (no hT dependency; PE can run them early)
                nc.tensor.matmul(grz, xt, wih[:, 0:2 * H], start=True,
                                 stop=(t == 0) and not has_gbias)
                gni_instant = (t == 0) and not has_gbias
                nc.tensor.matmul(gni, xt, wih[:, 2 * H:3 * H],
                                 start=True, stop=gni_instant)
                if has_gbias:
                    nc.tensor.matmul(grz, ones1[:, 0:BL], gbias[:, 0:2 * H],
                                     start=False, stop=(t == 0))
                    nc.tensor.matmul(gni, ones1[:, 0:BL], gbias[:, 2 * H:3 * H],
                                     start=False, stop=(t == 0))
                # h contributions
                if t > 0:
                    nc.tensor.matmul(grz, hT[:, 0:128], whh0[:, 0:2 * H],
                                     start=False, stop=False)
                    nc.tensor.matmul(grz, hT[:, 128:256], whh1[:, 0:2 * H],
                                     start=False, stop=True)
                    nc.tensor.matmul(gnh, hT[:, 0:128], whh0[:, 2 * H:3 * H],
                                     start=True, stop=False)
                    nc.tensor.matmul(gnh, hT[:, 128:256], whh1[:, 2 * H:3 * H],
                                     start=False, stop=True)

                r = wk.tile([BL, H], bf16, tag="r")
                nc.scalar.activation(r, grz[:, 0:H], AF.Sigmoid)
                z = wk.tile([BL, H], bf16, tag="z")
                nc.scalar.activation(z, grz[:, H:2 * H], AF.Sigmoid,
                                     bias=zbias[:, t:t + 1])
                zb = wk.tile([BL, H], bf16, tag="zb")
                nc.vector.scalar_tensor_tensor(zb, z, -1.0, onesb,
                                               op0=ALU.mult, op1=ALU.add)
                hTp = psT.tile([128, 2 * BL], f32, tag="hTp")
                if t > 0:
                    w1t = wk.tile([BL, H], f32, tag="w1t")
                    nc.gpsimd.tensor_mul(w1t, z, h_prev)
                    # transpose w1t into the hT accumulator early (off-chain)
                    nc.tensor.matmul(hTp[:, 0:BL], w1t[:, 0:128], eye32,
                                     is_transpose=True, start=True, stop=False)
                    nc.tensor.matmul(hTp[:, BL:2 * BL], w1t[:, 128:256], eye32,
                                     is_transpose=True, start=False, stop=False)

                n = wk.tile([BL, H], bf16, tag="n")
                if t > 0:
                    u = wk.tile([BL, H], bf16, tag="u")
                    nc.vector.tensor_mul(u, r, gnh)
                    nc.tensor.matmul(gni, eye16, u, start=False, stop=True)
                nc.scalar.activation(n, gni, AF.Tanh)

                w2t = wk.tile([BL, H], f32, tag="w2t")
                nc.vector.tensor_mul(w2t, zb, n)
                # h^T = T(w1t) + T(w2t) accumulated on PE
                nc.tensor.matmul(hTp[:, 0:BL], w2t[:, 0:128], eye32,
                                 is_transpose=True, start=(t == 0), stop=False)
                nc.tensor.matmul(hTp[:, BL:2 * BL], w2t[:, 128:256], eye32,
                                 is_transpose=True, start=False, stop=True)
                hT = wk.tile([128, 2 * BL], bf16, tag="hT")
                nc.vector.tensor_copy(hT[:, 0:BL], hTp[:, 0:BL])
                nc.vector.tensor_copy(hT[:, BL:2 * BL], hTp[:, BL:2 * BL])

                # h natural (off the recurrence chain), stored bf16 in the ring
                h = hring.tile([BL, H], bf16, tag="hb")
                if t > 0:
                    nc.vector.tensor_add(h, w1t, w2t)
                else:
                    nc.vector.tensor_copy(h, w2t)
                h_prev = h
                h_hist.append(h)

                # deferred attention/output block for the previous step
                if t > 0:
                    attn_out_block(t - 1, hT_prev_attn)
                hT_prev_attn = hT


            attn_out_block(L - 1, hT)

            # ---- deferred attention context ----
            Et = cp.tile([BL, L], f32, tag="Et")
            nc.scalar.activation(Et, scores, AF.Exp)
            ep = cp.tile([BL, L], f32, tag="ep")
            nc.vector.tensor_mul(ep, Et, maskt)
            d0 = cp.tile([BL, 1], f32, tag="d0")
            nc.vector.tensor_reduce(d0, ep, axis=mybir.AxisListType.X, op=ALU.add)
            dsum = cp.tile([BL, 1], f32, tag="dsum")
            nc.vector.tensor_add(dsum, d0, padd)
            rd = cp.tile([BL, 1], f32, tag="rd")
            nc.vector.reciprocal(rd, dsum)

            acc = psAcc.tile([BL, H], f32, tag="acc")
            for t in range(L):
                tmp = wk.tile([BL, H], bf16, tag="tmp")
                # split the 256 scalar-muls across DVE and GPSIMD (~2:1 rate)
                eng = nc.gpsimd if t % 3 == 2 else nc.vector
                eng.tensor_scalar_mul(tmp, h_hist[t], ep[:, t:t + 1])
                nc.tensor.matmul(acc, eye16, tmp, start=(t == 0), stop=(t == L - 1))

            ctx = cp.tile([BL, H], f32, tag="ctx")
            nc.vector.tensor_scalar_mul(ctx, acc, rd)

            # ---- cause MLP ----  (PSUM tiles reuse loop tags to stay in 8 banks)
            ctxTp = psS1.tile([128, 2 * BL], f32, tag="s1")
            nc.tensor.transpose(ctxTp[:, 0:BL], ctx[:, 0:128], eye32)
            nc.tensor.transpose(ctxTp[:, BL:2 * BL], ctx[:, 128:256], eye32)
            ctxT = cp.tile([128, 2 * BL], f32, tag="ctxT")
            nc.vector.tensor_copy(ctxT, ctxTp)

            zcp = psGrz.tile([BL, CH], f32, tag="grz")
            nc.tensor.matmul(zcp, ctxT[:, 0:BL], w1p[:, 0:CH], start=True, stop=False)
            nc.tensor.matmul(zcp, ctxT[:, BL:2 * BL], w1p[:, CH:2 * CH],
                             start=False, stop=False)
            nc.tensor.matmul(zcp, lastT, w1p[:, 2 * CH:3 * CH], start=False, stop=False)
            nc.tensor.matmul(zcp, ones1[:, 0:BL], b1r, start=False, stop=True)
            zc = cp.tile([BL, CH], f32, tag="zc")
            nc.scalar.activation(zc, zcp, AF.Relu)

            zcTp = psS1.tile([128, 2 * BL], f32, tag="s1")
            nc.tensor.transpose(zcTp[:, 0:BL], zc[:, 0:128], eye32)
            nc.tensor.transpose(zcTp[:, BL:2 * BL], zc[:, 128:256], eye32)
            zcT = cp.tile([128, 2 * BL], f32, tag="zcT")
            nc.vector.tensor_copy(zcT, zcTp)

            lg = psGrz.tile([BL, OUT], f32, tag="grz")
            nc.tensor.matmul(lg, zcT[:, 0:BL], w2p[:, 0:OUT], start=True, stop=False)
            nc.tensor.matmul(lg, zcT[:, BL:2 * BL], w2p[:, OUT:2 * OUT],
                             start=False, stop=False)
            nc.tensor.matmul(lg, ones1[:, 0:BL], b2r, start=False, stop=True)

            mx = cp.tile([BL, 1], f32, tag="mx")
            nc.vector.tensor_reduce(mx, lg, axis=mybir.AxisListType.X, op=ALU.max)
            nmx = cp.tile([BL, 1], f32, tag="nmx")
            nc.vector.tensor_scalar_mul(nmx, mx, -1.0)
            ex = cp.tile([BL, OUT], f32, tag="ex")
            sm = cp.tile([BL, 1], f32, tag="sm")
            nc.scalar.activation(ex, lg, AF.Exp, bias=nmx, accum_out=sm)
            rs = cp.tile([BL, 1], f32, tag="rs")
            nc.vector.reciprocal(rs, sm)
            fht = cp.tile([BL, OUT], f32, tag="fht")
            nc.vector.tensor_scalar_mul(fht, ex, rs)
            nc.sync.dma_start(out=d_fht[:], in_=fht)

    nc.finalize()
    _CACHE[key] = nc
    return nc


def _prep(input_batch, tte, W_ih, W_hh, b_ih, b_hh, W_out, b_out,
          Wa, Ua, va, W1, b1, W2, b2):
    input_batch = np.asarray(input_batch, np.float32)
    tte = np.asarray(tte, np.int32)
    W_ih = np.asarray(W_ih, np.float32)
    W_hh = np.asarray(W_hh, np.float32)
    b_ih = np.asarray(b_ih, np.float32)
    b_hh = np.asarray(b_hh, np.float32)
    W_out = np.asarray(W_out, np.float32)
    b_out = np.asarray(b_out, np.float32)
    Wa_ = np.asarray(Wa, np.float32)
    Ua = np.asarray(Ua, np.float32)
    va = np.asarray(va, np.float32)
    W1 = np.asarray(W1, np.float32)
    b1 = np.asarray(b1, np.float32)
    W2 = np.asarray(W2, np.float32)
    b2 = np.asarray(b2, np.float32)

    gb = b_ih + b_hh
    has_gbias = bool(np.any(gb))
    has_obias = bool(np.any(b_out))

    # shared (replicated) tensors
    wih_np = np.ascontiguousarray(W_ih.T).astype(BF16)              # [F, 3H]
    WhhT = W_hh.T                                                   # [H, 3H]
    whh_np = np.stack([WhhT[0:128], WhhT[128:256]]).astype(BF16)    # [2,128,3H]
    wa_np = np.concatenate([Wa_[0:128], Wa_[128:256]], axis=1).astype(BF16)
    WoutT = W_out.T                                                 # [H, F]
    woutT_np = np.concatenate([WoutT[0:128], WoutT[128:256]], axis=1).astype(BF16)
    w1_np = np.concatenate([W1[0:128], W1[128:256], W1[256:384]],
                           axis=1).astype(np.float32)               # [128, 3CH]
    w2_np = np.concatenate([W2[0:128], W2[128:256]], axis=1).astype(np.float32)
    b1_np = b1[None, :].astype(np.float32)
    b2_np = b2[None, :].astype(np.float32)
    eye16_np = np.eye(128, dtype=np.float32).astype(BF16)
    eye32_np = np.eye(128, dtype=np.float32)
    vab_np = np.broadcast_to(va, (BL, A)).astype(BF16).copy()
    gb_np = gb[None, :].astype(np.float32)
    ob_np = b_out[None, :].astype(np.float32)

    t_idx = np.arange(L, dtype=np.int32)[None, :]
    in_maps = []
    for c in range(NCORES):
        sl = slice(c * BL, (c + 1) * BL)
        xb = input_batch[sl]                      # [BL, L, F]
        ttec = tte[sl]
        last = xb[np.arange(BL), ttec]            # [BL, F]
        lastUa = last @ Ua                        # [BL, A]
        c_pad = np.tanh(lastUa) @ va              # [BL]
        pad_d = ((L - ttec).astype(np.float32) * np.exp(c_pad)).astype(np.float32)
        active = (t_idx < ttec[:, None]).astype(np.float32)  # [BL, L]
        m = {
            "xT": np.ascontiguousarray(xb.transpose(1, 2, 0)).astype(BF16),
            "wih": wih_np, "whh": whh_np, "wa": wa_np, "woutT": woutT_np,
            "lastua": lastUa.astype(BF16), "vab": vab_np,
            "mask": active, "zbias": (BIG * (1.0 - active)).astype(np.float32),
            "padd": pad_d[:, None],
            "lastT": np.ascontiguousarray(last.T).astype(np.float32),
            "w1p": w1_np, "w2p": w2_np, "b1r": b1_np, "b2r": b2_np,
            "eye16": eye16_np, "eye32": eye32_np,
        }
        if has_gbias:
            m["gbias"] = gb_np
        if has_obias:
            m["obias"] = ob_np
        in_maps.append(m)

    return in_maps, has_gbias, has_obias


def _prep_inputs(inputs):
    """Host-side preprocessing: returns (in_maps, has_gbias, has_obias)."""
    return _prep(**inputs)


def kernel(**inputs):
    in_maps, has_gbias, has_obias = _prep(**inputs)
    nc = _build(has_gbias, has_obias)
    res = run_bass_kernel_spmd(nc, in_maps, core_ids=list(range(NCORES)))
    outs = res.results
    output_batch = np.concatenate([r["out"] for r in outs], axis=0)
    fht = np.concatenate([r["fht"] for r in outs], axis=0)
    return output_batch.astype(np.float32), fht.astype(np.float32)
